# revision 1
# baseline (speedup 1.0000x reference)
"""Trainium2 Bass kernel for nn_MultiHeadAttention_41936060678770.

LinBERT-style linear attention:
  qh/kh/vh = LN(x) @ W + b  (per-stream LN, 16 heads x 64 dim)
  phi = elu(.)+1 ;  phi_k masked
  kv = sum_s phi_k (x) vh ; z = sum_s phi_k
  attn = (phi_q @ kv) / (phi_q @ z + eps)
  out = q + attn @ fc_w + fc_b

Sharding: 8 cores, tokens split 8-ways over flattened (B*S); each pair of
cores (2c, 2c+1) holds one batch, so the [16,64,65] kv/z state is
all-reduced within core pairs; everything else is fully local.

Layout strategy per core (2048 tokens, 16 tiles of 128):
  - activations live tokens-on-partitions ("natural"); contraction operands
    are produced by XBAR DMA-transpose of bf16 tiles (2-byte only, free of
    PE/DVE cost).
  - LN is folded to one fused DVE op: xn = (x - mu) * rsig  (g==1, b==0
    fast path verified on host; general path scales W rows by g on device
    and adds c = b@W + b_proj via a broadcast tile).
  - All big matmuls run in bf16 with fp32 PSUM accumulation.
  - kv state and z accumulate in PSUM across all 16 token tiles
    ([64,65] per head packed 8 heads/bank x 2 banks), then AllReduce.
"""
import sys

sys.path.insert(0, "/opt/trn_rl_repo")

import numpy as np

import concourse.bacc as bacc
import concourse.bass as bass
import concourse.tile as tile
import concourse.mybir as mybir
from concourse.bass_utils import run_bass_kernel_spmd

F32 = mybir.dt.float32
BF16 = mybir.dt.bfloat16
AF = mybir.ActivationFunctionType
ALU = mybir.AluOpType

B, S, HS = 4, 4096, 1024
NH, D = 16, 64
NCORES = 8
TOK = B * S // NCORES          # 2048 rows per core
NT = TOK // 128                # 16 token tiles
KT = HS // 128                 # 8 hidden tiles
LN_EPS = 1e-5
ATT_EPS = 1e-6


def _ln_project(nc, pool, psp, stat_pool, eps_t, x_nat, w_sb, c_bcast):
    """LN-center+scale -> transpose -> project. Returns list of 2 psum tiles
    [128,512] (the projected output chunks, pre-activation) plus the fused
    normalized-transposed tile so callers can keep references alive."""
    mv = stat_pool.tile([128, 2], F32, tag="mv")
    stats = stat_pool.tile([128, 2, 6], F32, tag="stats")
    nc.vector.bn_stats(out=stats[:, 0, :], in_=x_nat[:, 0:512])
    nc.vector.bn_stats(out=stats[:, 1, :], in_=x_nat[:, 512:1024])
    nc.vector.bn_aggr(out=mv[:], in_=stats[:])
    # rsig = rsqrt(var+eps) fully on DVE (quake seed + 2 Newton steps) —
    # ACT then only ever runs Exp/Copy (one table set, no ~1.3us
    # ACT_TABLE_LOAD thrash per LN<->elu switch).
    veps = stat_pool.tile([128, 1], F32, tag="veps")
    nc.vector.tensor_scalar_add(out=veps[:], in0=mv[:, 1:2], scalar1=LN_EPS)
    seed = stat_pool.tile([128, 1], mybir.dt.int32, tag="seed")
    nc.vector.tensor_scalar(
        out=seed[:], in0=veps[:].bitcast(mybir.dt.int32),
        scalar1=1, scalar2=None, op0=ALU.arith_shift_right)
    nc.vector.tensor_scalar(
        out=seed[:], in0=seed[:], scalar1=-1, scalar2=0x5F3759DF,
        op0=ALU.mult, op1=ALU.add)
    y0 = seed[:].bitcast(F32)
    t_nr = stat_pool.tile([128, 1], F32, tag="t_nr")
    sig = stat_pool.tile([128, 1], F32, tag="sig")
    nc.vector.tensor_tensor(out=t_nr[:], in0=y0, in1=y0, op=ALU.mult)
    nc.vector.tensor_tensor(out=t_nr[:], in0=t_nr[:], in1=veps[:],
                            op=ALU.mult)
    nc.vector.tensor_scalar(out=t_nr[:], in0=t_nr[:], scalar1=-0.5,
                            scalar2=1.5, op0=ALU.mult, op1=ALU.add)
    nc.vector.tensor_tensor(out=sig[:], in0=y0, in1=t_nr[:], op=ALU.mult)
    nc.vector.tensor_tensor(out=t_nr[:], in0=sig[:], in1=sig[:], op=ALU.mult)
    nc.vector.tensor_tensor(out=t_nr[:], in0=t_nr[:], in1=veps[:],
                            op=ALU.mult)
    nc.vector.tensor_scalar(out=t_nr[:], in0=t_nr[:], scalar1=-0.5,
                            scalar2=1.5, op0=ALU.mult, op1=ALU.add)
    nc.vector.tensor_tensor(out=sig[:], in0=sig[:], in1=t_nr[:], op=ALU.mult)
    xn = pool.tile([128, HS], BF16, tag="xn")
    nc.vector.tensor_scalar(
        out=xn[:], in0=x_nat[:], scalar1=mv[:, 0:1], scalar2=sig[:],
        op0=ALU.subtract, op1=ALU.mult,
    )
    xnT = pool.tile([128, KT, 128], BF16, tag="xnT")
    nc.sync.dma_start_transpose(out=xnT[:], in_=xn[:])

    ps_chunks = []
    for c in range(2):
        ps = psp.tile([128, 512], F32, tag="proj")
        for kt in range(KT):
            nc.tensor.matmul(
                ps[:], xnT[:, kt, :], w_sb[:, kt, c * 512:(c + 1) * 512],
                start=(kt == 0), stop=(kt == KT - 1),
            )
        if c_bcast is not None:
            nc.vector.tensor_tensor(
                out=ps[:], in0=ps[:], in1=c_bcast[:, c * 512:(c + 1) * 512],
                op=ALU.add,
            )
        ps_chunks.append(ps)
    return ps_chunks, xnT


def _elu1(nc, pool, src, out_ap, mask_col):
    """out = elu(src)+1 = exp(min(src,0)) + max(src,0), optionally * mask."""
    tmin = pool.tile([128, 512], F32, tag="tmin")
    nc.vector.tensor_scalar_min(out=tmin[:], in0=src[:], scalar1=0.0)
    texp = pool.tile([128, 512], F32, tag="texp")
    nc.scalar.activation(out=texp[:], in_=tmin[:], func=AF.Exp)
    if mask_col is None:
        nc.vector.scalar_tensor_tensor(
            out=out_ap, in0=src[:], scalar=0.0, in1=texp[:],
            op0=ALU.max, op1=ALU.add,
        )
    else:
        tphi = pool.tile([128, 512], F32, tag="tphi")
        nc.vector.scalar_tensor_tensor(
            out=tphi[:], in0=src[:], scalar=0.0, in1=texp[:],
            op0=ALU.max, op1=ALU.add,
        )
        nc.vector.tensor_scalar_mul(out=out_ap, in0=tphi[:], scalar1=mask_col)


def build(g_trivial: bool, c_trivial: bool, mask_trivial: bool,
          replica_groups, _skip_collective=False):
    nc = bacc.Bacc(None)

    qx_d = nc.dram_tensor("qx", [TOK, HS], F32, kind="ExternalInput")
    kx_d = nc.dram_tensor("kx", [TOK, HS], F32, kind="ExternalInput")
    vx_d = nc.dram_tensor("vx", [TOK, HS], F32, kind="ExternalInput")
    w_d = {
        "q": nc.dram_tensor("wq", [HS, HS], F32, kind="ExternalInput"),
        "k": nc.dram_tensor("wk", [HS, HS], F32, kind="ExternalInput"),
        "v": nc.dram_tensor("wv", [HS, HS], F32, kind="ExternalInput"),
        "fc": nc.dram_tensor("fcw", [HS, HS], F32, kind="ExternalInput"),
    }
    if not mask_trivial:
        mask_d = nc.dram_tensor("maskx", [TOK, 1], F32, kind="ExternalInput")
    if not g_trivial:
        g_d = {s: nc.dram_tensor(f"g_{s}", [HS], F32, kind="ExternalInput")
               for s in ("q", "k", "v")}
    if not c_trivial:
        # c vectors are computed on host?? no: computed on device from b/bias
        b_d = {s: nc.dram_tensor(f"b_{s}", [HS], F32, kind="ExternalInput")
               for s in ("q", "k", "v")}
        pb_d = {s: nc.dram_tensor(f"pb_{s}", [HS], F32, kind="ExternalInput")
                for s in ("q", "k", "v")}
        fcb_d = nc.dram_tensor("fcb", [HS], F32, kind="ExternalInput")

    out_d = nc.dram_tensor("out", [TOK, HS], F32, kind="ExternalOutput")

    from contextlib import ExitStack
    with tile.TileContext(nc) as tc, ExitStack() as ctx:
        wpool = ctx.enter_context(tc.tile_pool(name="weights", bufs=1))
        consts = ctx.enter_context(tc.tile_pool(name="consts", bufs=1))
        proj_ps = ctx.enter_context(
            tc.tile_pool(name="proj_ps", bufs=4, space="PSUM"))
        dram_p = ctx.enter_context(
            tc.tile_pool(name="dram", bufs=1, space="DRAM"))

        eps_t = consts.tile([128, 1], F32)
        nc.vector.memset(eps_t[:], LN_EPS)
        att_eps_t = consts.tile([128, 1], F32)
        nc.vector.memset(att_eps_t[:], ATT_EPS)

        # ---------------- weights ----------------
        w_sb = {}
        for s in ("q", "k", "v", "fc"):
            w_sb[s] = wpool.tile([128, KT, HS], BF16, tag=f"w_{s}", name=f"w_{s}")
            src = w_d[s].rearrange("(kt p) n -> p kt n", p=128)
            if g_trivial or s == "fc":
                nc.gpsimd.dma_start(out=w_sb[s][:], in_=src)
            else:
                wtmp = consts.tile([128, KT, HS], F32, tag="wtmp")
                nc.sync.dma_start(out=wtmp[:], in_=src)
                g_sb = consts.tile([128, KT], F32, tag=f"gsb_{s}")
                nc.sync.dma_start(
                    out=g_sb[:], in_=g_d[s].rearrange("(kt p) -> p kt", p=128))
                for kt in range(KT):
                    nc.vector.tensor_scalar_mul(
                        out=w_sb[s][:, kt, :], in0=wtmp[:, kt, :],
                        scalar1=g_sb[:, kt:kt + 1])

        # c = b @ (g*W) + proj_bias, broadcast across partitions
        c_bc = {"q": None, "k": None, "v": None}
        fcb_bc = None
        if not c_trivial:
            for s in ("q", "k", "v"):
                b_bf = consts.tile([128, KT], BF16, tag=f"bbf_{s}")
                nc.gpsimd.dma_start(
                    out=b_bf[:], in_=b_d[s].rearrange("(kt p) -> p kt", p=128))
                cps = proj_ps.tile([1, HS], F32, tag="c_ps")
                for kt in range(KT):
                    nc.tensor.matmul(cps[:], b_bf[:, kt:kt + 1],
                                     w_sb[s][:, kt, :],
                                     start=(kt == 0), stop=(kt == KT - 1))
                crow = consts.tile([1, HS], F32, tag=f"crow_{s}")
                pbrow = consts.tile([1, HS], F32, tag=f"pbrow_{s}")
                nc.sync.dma_start(out=pbrow[:], in_=pb_d[s][None, :])
                nc.vector.tensor_tensor(out=crow[:], in0=cps[:], in1=pbrow[:],
                                        op=ALU.add)
                c_bc[s] = consts.tile([128, HS], F32, tag=f"cbc_{s}", name=f"cbc_{s}")
                nc.gpsimd.partition_broadcast(c_bc[s][:], crow[:])
            fcb_row = consts.tile([1, HS], F32, tag="fcb_row")
            nc.sync.dma_start(out=fcb_row[:], in_=fcb_d[None, :])
            fcb_bc = consts.tile([128, HS], F32, tag="fcb_bc")
            nc.gpsimd.partition_broadcast(fcb_bc[:], fcb_row[:])

        # ---------------- sweep 1: K/V + kv state ----------------
        kv_sb = consts.tile([128, 8, D + 1], F32, tag="kv_sb")
        with (
            tc.tile_pool(name="kv_ps", bufs=1, space="PSUM") as kv_psp,
            tc.tile_pool(name="s1", bufs=4) as s1,
            tc.tile_pool(name="stat1", bufs=8) as stat1,
        ):
            kv_ps = [kv_psp.tile([128, 4, D + 1], F32, tag=f"kv{b}", name=f"kv{b}",
                                 padded_shape=[128, 4, 128])
                     for b in range(2)]
            for i in range(NT):
                r0 = i * 128
                k_nat = s1.tile([128, HS], BF16, tag="k_nat")
                nc.gpsimd.dma_start(out=k_nat[:], in_=kx_d[r0:r0 + 128, :])
                v_nat = s1.tile([128, HS], BF16, tag="v_nat")
                nc.gpsimd.dma_start(out=v_nat[:], in_=vx_d[r0:r0 + 128, :])
                mask_col = None
                if not mask_trivial:
                    mcol = stat1.tile([128, 1], F32, tag="mcol")
                    nc.sync.dma_start(out=mcol[:], in_=mask_d[r0:r0 + 128, :])
                    mask_col = mcol[:]

                kh_ps, _knT = _ln_project(nc, s1, proj_ps, stat1, eps_t,
                                          k_nat, w_sb["k"], c_bc["k"])
                phi_k = s1.tile([128, HS], BF16, tag="phi_k")
                for c in range(2):
                    _elu1(nc, s1, kh_ps[c], phi_k[:, c * 512:(c + 1) * 512],
                          mask_col)

                vh_ps, _vnT = _ln_project(nc, s1, proj_ps, stat1, eps_t,
                                          v_nat, w_sb["v"], c_bc["v"])
                vh_aug = s1.tile([128, NH, D + 1], BF16, tag="vh_aug")
                nc.vector.memset(vh_aug[:, :, D:D + 1], 1.0)
                for c in range(2):
                    nc.vector.tensor_copy(
                        out=vh_aug[:, c * 8:(c + 1) * 8, 0:D],
                        in_=vh_ps[c][:].rearrange("p (n d) -> p n d", d=D))

                for n in range(NH):
                    beta, j, hs = n // 8, (n // 2) % 4, (n % 2) * 64
                    nc.tensor.matmul(
                        kv_ps[beta][hs:hs + 64, j, :],
                        phi_k[:, n * D:(n + 1) * D],
                        vh_aug[:, n, :],
                        start=(i == 0), stop=(i == NT - 1),
                        tile_position=(0, hs),
                        skip_group_check=True,
                    )

            nc.vector.tensor_copy(out=kv_sb[:, 0:4, :], in_=kv_ps[0][:])
            nc.vector.tensor_copy(out=kv_sb[:, 4:8, :], in_=kv_ps[1][:])

        # ---------------- all-reduce kv state within batch pairs ----------
        # kv2 holds the reduced state as 8 block-diagonal [128, 130] bf16
        # operands (head-pair 2m/2m+1), so the num/den matmul is a plain
        # K=128 matmul at base partition 0 (operand base_partition=64
        # matmuls fault the exec unit on hardware).
        kv2 = consts.tile([128, 8, 2 * (D + 1)], BF16, tag="kv2")
        nc.vector.memset(kv2[:], 0.0)
        if _skip_collective:
            nc.vector.tensor_copy(out=kv2[0:64, :, 0:D + 1],
                                  in_=kv_sb[0:64, :, :])
            nc.vector.tensor_copy(out=kv2[64:128, :, D + 1:2 * (D + 1)],
                                  in_=kv_sb[64:128, :, :])
        else:
            cc_in = dram_p.tile([128, 8, D + 1], F32)
            cc_out = dram_p.tile([128, 8, D + 1], F32)
            nc.gpsimd.dma_start(out=cc_in[:], in_=kv_sb[:])
            nc.gpsimd.collective_compute(
                "AllReduce", ALU.add, replica_groups=replica_groups,
                ins=[cc_in.opt()], outs=[cc_out.opt()],
            )
            nc.gpsimd.dma_start(out=kv2[0:64, :, 0:D + 1],
                                in_=cc_out[0:64, :, :])
            nc.gpsimd.dma_start(out=kv2[64:128, :, D + 1:2 * (D + 1)],
                                in_=cc_out[64:128, :, :])

        # ---------------- sweep 2: Q -> attn -> fc -> out ----------------
        with (
            tc.tile_pool(name="nd_ps", bufs=4, space="PSUM") as nd_psp,
            tc.tile_pool(name="s2", bufs=4) as s2,
            tc.tile_pool(name="stat2", bufs=8) as stat2,
        ):
            for i in range(NT):
                r0 = i * 128
                q_nat = s2.tile([128, HS], F32, tag="q_nat")
                nc.gpsimd.dma_start(out=q_nat[:], in_=qx_d[r0:r0 + 128, :])

                qh_ps, _qnT = _ln_project(nc, s2, proj_ps, stat2, eps_t,
                                          q_nat, w_sb["q"], c_bc["q"])
                phi_q = s2.tile([128, HS], BF16, tag="phi_q")
                for c in range(2):
                    _elu1(nc, s2, qh_ps[c], phi_q[:, c * 512:(c + 1) * 512],
                          None)
                phi_qT = s2.tile([128, KT, 128], BF16, tag="phi_qT")
                nc.sync.dma_start_transpose(out=phi_qT[:], in_=phi_q[:])

                attn = s2.tile([128, HS], BF16, tag="attn")
                # 2 head-pairs per PSUM bank; den processed batched per tile
                nds = []
                den = stat2.tile([128, NH], F32, tag="den")
                for m in range(8):       # head pair (2m, 2m+1) per matmul
                    if m % 2 == 0:
                        nd2 = nd_psp.tile([128, 2, 2 * (D + 1)], F32,
                                          tag="nd", name="nd",
                                          padded_shape=[128, 2, 256])
                        nds.append(nd2)
                    nd = nd2[:, m % 2, :]
                    nc.tensor.matmul(
                        nd, phi_qT[:, m, :], kv2[:, m, :],
                        start=True, stop=True,
                    )
                    nc.vector.tensor_copy(out=den[:, 2 * m:2 * m + 2],
                                          in_=nd[:, D::D + 1])
                rd = stat2.tile([128, NH], F32, tag="rd")
                nc.vector.tensor_scalar_add(out=rd[:], in0=den[:],
                                            scalar1=ATT_EPS)
                nc.vector.reciprocal(out=rd[:], in_=rd[:])
                for n in range(NH):
                    nd = nds[n // 4][:, (n // 2) % 2, :]
                    nc.scalar.activation(
                        out=attn[:, n * D:(n + 1) * D],
                        in_=nd[:, (n % 2) * (D + 1):(n % 2) * (D + 1) + D],
                        func=AF.Copy, bias=0.0, scale=rd[:, n:n + 1])

                attnT = s2.tile([128, KT, 128], BF16, tag="attnT")
                nc.sync.dma_start_transpose(out=attnT[:], in_=attn[:])

                out_sb = s2.tile([128, HS], F32, tag="out_sb")
                for c in range(2):
                    ps = proj_ps.tile([128, 512], F32, tag="proj")
                    for kt in range(KT):
                        nc.tensor.matmul(
                            ps[:], attnT[:, kt, :],
                            w_sb["fc"][:, kt, c * 512:(c + 1) * 512],
                            start=(kt == 0), stop=(kt == KT - 1))
                    if fcb_bc is not None:
                        nc.vector.tensor_tensor(
                            out=ps[:], in0=ps[:],
                            in1=fcb_bc[:, c * 512:(c + 1) * 512], op=ALU.add)
                    nc.vector.tensor_tensor(
                        out=out_sb[:, c * 512:(c + 1) * 512], in0=ps[:],
                        in1=q_nat[:, c * 512:(c + 1) * 512], op=ALU.add)
                nc.gpsimd.dma_start(out=out_d[r0:r0 + 128, :], in_=out_sb[:])

    nc.compile()
    return nc


_BUILD_CACHE = {}


def _get_nc(flags, replica_groups):
    key = (flags, tuple(tuple(g) for g in replica_groups))
    if key not in _BUILD_CACHE:
        _BUILD_CACHE[key] = build(*flags, replica_groups)
    return _BUILD_CACHE[key]


def kernel(q, k, v, ln_q_g, ln_q_b, wq, bq, ln_k_g, ln_k_b, wk, bk,
           ln_v_g, ln_v_b, wv, bv, fc_w, fc_b, mask):
    q = np.ascontiguousarray(q, np.float32).reshape(B * S, HS)
    k = np.ascontiguousarray(k, np.float32).reshape(B * S, HS)
    v = np.ascontiguousarray(v, np.float32).reshape(B * S, HS)
    mask_f = np.ascontiguousarray(mask, np.float32).reshape(B * S, 1)
    wq = np.ascontiguousarray(wq, np.float32)
    wk = np.ascontiguousarray(wk, np.float32)
    wv = np.ascontiguousarray(wv, np.float32)
    fc_w = np.ascontiguousarray(fc_w, np.float32)

    g_trivial = all(np.all(x == 1.0) for x in (ln_q_g, ln_k_g, ln_v_g))
    c_trivial = all(np.all(x == 0.0) for x in
                    (ln_q_b, ln_k_b, ln_v_b, bq, bk, bv, fc_b))
    mask_trivial = bool(np.all(mask_f == 1.0))

    groups = [[0, 1], [2, 3], [4, 5], [6, 7]]
    nc = _get_nc((g_trivial, c_trivial, mask_trivial), groups)

    in_maps = []
    for c in range(NCORES):
        r0, r1 = c * TOK, (c + 1) * TOK
        m = {
            "qx": q[r0:r1], "kx": k[r0:r1], "vx": v[r0:r1],
            "wq": wq, "wk": wk, "wv": wv, "fcw": fc_w,
        }
        if not mask_trivial:
            m["maskx"] = mask_f[r0:r1]
        if not g_trivial:
            m.update({"g_q": np.asarray(ln_q_g, np.float32),
                      "g_k": np.asarray(ln_k_g, np.float32),
                      "g_v": np.asarray(ln_v_g, np.float32)})
        if not c_trivial:
            m.update({"b_q": np.asarray(ln_q_b, np.float32),
                      "b_k": np.asarray(ln_k_b, np.float32),
                      "b_v": np.asarray(ln_v_b, np.float32),
                      "pb_q": np.asarray(bq, np.float32),
                      "pb_k": np.asarray(bk, np.float32),
                      "pb_v": np.asarray(bv, np.float32),
                      "fcb": np.asarray(fc_b, np.float32)})
        in_maps.append(m)

    res = run_bass_kernel_spmd(nc, in_maps, list(range(NCORES)))
    out = np.concatenate([res.results[c]["out"] for c in range(NCORES)], 0)
    return out.reshape(B, S, HS).astype(np.float32)



# revision 3
# speedup vs baseline: 1.4175x; 1.4175x over previous
"""Trainium2 Bass kernel for nn_MultiHeadAttention_41936060678770.

LinBERT-style linear attention:
  qh/kh/vh = LN(x) @ W + b  (per-stream LN, 16 heads x 64 dim)
  phi = elu(.)+1 ;  phi_k masked
  kv = sum_s phi_k (x) vh ; z = sum_s phi_k
  attn = (phi_q @ kv) / (phi_q @ z + eps)
  out = q + attn @ fc_w + fc_b

Sharding: 8 cores, tokens split 8-ways over flattened (B*S); each pair of
cores (2c, 2c+1) holds one batch, so the [16,64,65] kv/z state is
all-reduced within core pairs; everything else is fully local.

v2 perf notes (from the 745us baseline trace): the PE ran at its mid
p-state (~1.2GHz) because dependency stalls kept resetting the 3us
continuous-busy ramp to 2.4GHz, and engine queues serialized loads
behind transposes. This version:
  - software-pipelines both sweeps (stage A for tile i+1 emitted while
    tile i's matmuls stream) so the PE queue is always 1+ tile ahead;
  - moves the LN apply to ACT (Identity with AP scale/bias), elu's exp
    reads PSUM directly (elu1 = max(x,0)+min(exp(x),1)), vh_aug copies
    and attn scaling also on ACT; DVE keeps stats/newton/min/stt;
  - activation loads/stores go through the sync HWDGE queue (f32, no
    gpsimd SWDGE overhead); weights stream chunk-by-chunk on gpsimd so
    the first projection starts ~3us in (baseline had an 80us bubble);
  - Identity/Exp/Relu/Copy all live in one ACT table set (no reloads).
"""
import sys

sys.path.insert(0, "/opt/trn_rl_repo")

import numpy as np

import concourse.bacc as bacc
import concourse.bass as bass
import concourse.tile as tile
import concourse.mybir as mybir
from concourse.bass_utils import run_bass_kernel_spmd

F32 = mybir.dt.float32
BF16 = mybir.dt.bfloat16
AF = mybir.ActivationFunctionType
ALU = mybir.AluOpType

B, S, HS = 4, 4096, 1024
NH, D = 16, 64
NCORES = 8
TOK = B * S // NCORES          # 2048 rows per core
NT = TOK // 128                # 16 token tiles
KT = HS // 128                 # 8 hidden tiles
LN_EPS = 1e-5
ATT_EPS = 1e-6
PF = 2                         # load prefetch depth (tiles)


def _ln_stats(nc, stat_pool, x_nat):
    """bn stats + rsqrt(var+eps) via DVE newton. Returns (sig, negmusig)."""
    mv = stat_pool.tile([128, 2], F32, tag="mv")
    stats = stat_pool.tile([128, 2, 6], F32, tag="stats")
    nc.vector.bn_stats(out=stats[:, 0, :], in_=x_nat[:, 0:512])
    nc.vector.bn_stats(out=stats[:, 1, :], in_=x_nat[:, 512:1024])
    nc.vector.bn_aggr(out=mv[:], in_=stats[:])
    # rsig = rsqrt(var+eps) fully on DVE (quake seed + 2 Newton steps) so
    # ACT only ever runs one table set.
    veps = stat_pool.tile([128, 1], F32, tag="veps")
    nc.vector.tensor_scalar_add(out=veps[:], in0=mv[:, 1:2], scalar1=LN_EPS)
    seed = stat_pool.tile([128, 1], mybir.dt.int32, tag="seed")
    nc.vector.tensor_scalar(
        out=seed[:], in0=veps[:].bitcast(mybir.dt.int32),
        scalar1=1, scalar2=None, op0=ALU.arith_shift_right)
    nc.vector.tensor_scalar(
        out=seed[:], in0=seed[:], scalar1=-1, scalar2=0x5F3759DF,
        op0=ALU.mult, op1=ALU.add)
    y0 = seed[:].bitcast(F32)
    t_nr = stat_pool.tile([128, 1], F32, tag="t_nr")
    sig = stat_pool.tile([128, 1], F32, tag="sig")
    nc.vector.tensor_tensor(out=t_nr[:], in0=y0, in1=y0, op=ALU.mult)
    nc.vector.tensor_tensor(out=t_nr[:], in0=t_nr[:], in1=veps[:],
                            op=ALU.mult)
    nc.vector.tensor_scalar(out=t_nr[:], in0=t_nr[:], scalar1=-0.5,
                            scalar2=1.5, op0=ALU.mult, op1=ALU.add)
    nc.vector.tensor_tensor(out=sig[:], in0=y0, in1=t_nr[:], op=ALU.mult)
    nc.vector.tensor_tensor(out=t_nr[:], in0=sig[:], in1=sig[:], op=ALU.mult)
    nc.vector.tensor_tensor(out=t_nr[:], in0=t_nr[:], in1=veps[:],
                            op=ALU.mult)
    nc.vector.tensor_scalar(out=t_nr[:], in0=t_nr[:], scalar1=-0.5,
                            scalar2=1.5, op0=ALU.mult, op1=ALU.add)
    nc.vector.tensor_tensor(out=sig[:], in0=sig[:], in1=t_nr[:], op=ALU.mult)
    negmusig = stat_pool.tile([128, 1], F32, tag="negmusig")
    nc.vector.scalar_tensor_tensor(
        out=negmusig[:], in0=mv[:, 0:1], scalar=-1.0, in1=sig[:],
        op0=ALU.mult, op1=ALU.mult)
    return sig, negmusig


def _elu1(nc, pool, src_ps, out_ap, mask_col):
    """out = elu(src)+1 = max(src,0) + min(exp(src),1), optionally * mask.
    exp reads PSUM directly on ACT (values here never overflow exp)."""
    texp = pool.tile([128, 512], BF16, tag="texp")
    nc.scalar.activation(out=texp[:], in_=src_ps[:], func=AF.Exp)
    tmin = pool.tile([128, 512], BF16, tag="tmin")
    nc.vector.tensor_scalar_min(out=tmin[:], in0=texp[:], scalar1=1.0)
    if mask_col is None:
        nc.vector.scalar_tensor_tensor(
            out=out_ap, in0=src_ps[:], scalar=0.0, in1=tmin[:],
            op0=ALU.max, op1=ALU.add,
        )
    else:
        tphi = pool.tile([128, 512], F32, tag="tphi")
        nc.vector.scalar_tensor_tensor(
            out=tphi[:], in0=src_ps[:], scalar=0.0, in1=tmin[:],
            op0=ALU.max, op1=ALU.add,
        )
        nc.vector.tensor_scalar_mul(out=out_ap, in0=tphi[:], scalar1=mask_col)


def build(g_trivial: bool, c_trivial: bool, mask_trivial: bool,
          replica_groups, _skip_collective=False):
    nc = bacc.Bacc(None)

    qx_d = nc.dram_tensor("qx", [TOK, HS], F32, kind="ExternalInput")
    kx_d = nc.dram_tensor("kx", [TOK, HS], F32, kind="ExternalInput")
    vx_d = nc.dram_tensor("vx", [TOK, HS], F32, kind="ExternalInput")
    w_d = {
        "q": nc.dram_tensor("wq", [HS, HS], F32, kind="ExternalInput"),
        "k": nc.dram_tensor("wk", [HS, HS], F32, kind="ExternalInput"),
        "v": nc.dram_tensor("wv", [HS, HS], F32, kind="ExternalInput"),
        "fc": nc.dram_tensor("fcw", [HS, HS], F32, kind="ExternalInput"),
    }
    if not mask_trivial:
        mask_d = nc.dram_tensor("maskx", [TOK, 1], F32, kind="ExternalInput")
    if not g_trivial:
        g_d = {s: nc.dram_tensor(f"g_{s}", [HS], F32, kind="ExternalInput")
               for s in ("q", "k", "v")}
    if not c_trivial:
        b_d = {s: nc.dram_tensor(f"b_{s}", [HS], F32, kind="ExternalInput")
               for s in ("q", "k", "v")}
        pb_d = {s: nc.dram_tensor(f"pb_{s}", [HS], F32, kind="ExternalInput")
                for s in ("q", "k", "v")}
        fcb_d = nc.dram_tensor("fcb", [HS], F32, kind="ExternalInput")

    out_d = nc.dram_tensor("out", [TOK, HS], F32, kind="ExternalOutput")

    from contextlib import ExitStack
    with tile.TileContext(nc) as tc, ExitStack() as ctx:
        wpool = ctx.enter_context(tc.tile_pool(name="weights", bufs=1))
        consts = ctx.enter_context(tc.tile_pool(name="consts", bufs=1))
        proj_ps = ctx.enter_context(
            tc.tile_pool(name="proj_ps", bufs=4, space="PSUM"))
        dram_p = ctx.enter_context(
            tc.tile_pool(name="dram", bufs=1, space="DRAM"))

        # ---------------- weights (gpsimd cast DMA, chunked) ----------------
        w_sb = {}
        w_src = {}
        for s in ("q", "k", "v", "fc"):
            w_sb[s] = wpool.tile([128, KT, HS], BF16, tag=f"w_{s}",
                                 name=f"w_{s}")
            w_src[s] = w_d[s].rearrange("(kt p) n -> p kt n", p=128)
        if g_trivial:
            # interleave k/v chunks first so sweep1's first projections can
            # start as soon as chunk kt arrives; q/fc stream in behind.
            for kt in range(KT):
                nc.gpsimd.dma_start(out=w_sb["k"][:, kt, :],
                                    in_=w_src["k"][:, kt, :])
                nc.gpsimd.dma_start(out=w_sb["v"][:, kt, :],
                                    in_=w_src["v"][:, kt, :])
            for kt in range(KT):
                nc.gpsimd.dma_start(out=w_sb["q"][:, kt, :],
                                    in_=w_src["q"][:, kt, :])
            nc.gpsimd.dma_start(out=w_sb["fc"][:], in_=w_src["fc"])
        else:
            nc.gpsimd.dma_start(out=w_sb["fc"][:], in_=w_src["fc"])
            for s in ("k", "v", "q"):
                wtmp = consts.tile([128, KT, HS], F32, tag="wtmp")
                nc.sync.dma_start(out=wtmp[:], in_=w_src[s])
                g_sb = consts.tile([128, KT], F32, tag=f"gsb_{s}")
                nc.sync.dma_start(
                    out=g_sb[:], in_=g_d[s].rearrange("(kt p) -> p kt", p=128))
                for kt in range(KT):
                    nc.vector.tensor_scalar_mul(
                        out=w_sb[s][:, kt, :], in0=wtmp[:, kt, :],
                        scalar1=g_sb[:, kt:kt + 1])

        # c = b @ (g*W) + proj_bias, broadcast across partitions
        c_bc = {"q": None, "k": None, "v": None}
        fcb_bc = None
        if not c_trivial:
            for s in ("q", "k", "v"):
                b_bf = consts.tile([128, KT], BF16, tag=f"bbf_{s}")
                nc.gpsimd.dma_start(
                    out=b_bf[:], in_=b_d[s].rearrange("(kt p) -> p kt", p=128))
                cps = proj_ps.tile([1, HS], F32, tag="c_ps")
                for kt in range(KT):
                    nc.tensor.matmul(cps[:], b_bf[:, kt:kt + 1],
                                     w_sb[s][:, kt, :],
                                     start=(kt == 0), stop=(kt == KT - 1))
                crow = consts.tile([1, HS], F32, tag=f"crow_{s}")
                pbrow = consts.tile([1, HS], F32, tag=f"pbrow_{s}")
                nc.sync.dma_start(out=pbrow[:], in_=pb_d[s][None, :])
                nc.vector.tensor_tensor(out=crow[:], in0=cps[:], in1=pbrow[:],
                                        op=ALU.add)
                c_bc[s] = consts.tile([128, HS], F32, tag=f"cbc_{s}",
                                      name=f"cbc_{s}")
                nc.gpsimd.partition_broadcast(c_bc[s][:], crow[:])
            fcb_row = consts.tile([1, HS], F32, tag="fcb_row")
            nc.sync.dma_start(out=fcb_row[:], in_=fcb_d[None, :])
            fcb_bc = consts.tile([128, HS], F32, tag="fcb_bc")
            nc.gpsimd.partition_broadcast(fcb_bc[:], fcb_row[:])

        # ---------------- sweep 1: K/V + kv state ----------------
        kv_sb = consts.tile([128, 8, D + 1], F32, tag="kv_sb")
        with (
            tc.tile_pool(name="kv_ps", bufs=1, space="PSUM") as kv_psp,
            tc.tile_pool(name="s1", bufs=4) as s1,
            tc.tile_pool(name="ld1", bufs=4) as ld1,
            tc.tile_pool(name="stat1", bufs=8) as stat1,
        ):
            kv_ps = [kv_psp.tile([128, 4, D + 1], F32, tag=f"kv{b}",
                                 name=f"kv{b}", padded_shape=[128, 4, 128])
                     for b in range(2)]

            loads = {}

            def emit_load1(i):
                if i >= NT:
                    return
                r0 = i * 128
                k_nat = ld1.tile([128, HS], F32, tag="k_nat")
                nc.sync.dma_start(out=k_nat[:], in_=kx_d[r0:r0 + 128, :])
                v_nat = ld1.tile([128, HS], F32, tag="v_nat")
                nc.sync.dma_start(out=v_nat[:], in_=vx_d[r0:r0 + 128, :])
                mask_col = None
                if not mask_trivial:
                    mcol = stat1.tile([128, 1], F32, tag="mcol")
                    nc.sync.dma_start(out=mcol[:], in_=mask_d[r0:r0 + 128, :])
                    mask_col = mcol[:]
                loads[i] = (k_nat, v_nat, mask_col)

            def emit_A1(i):
                """LN + xn + transpose for k and v of tile i."""
                if i >= NT:
                    return None
                k_nat, v_nat, mask_col = loads.pop(i)
                res = {}
                for s, x_nat in (("k", k_nat), ("v", v_nat)):
                    sig, negmusig = _ln_stats(nc, stat1, x_nat)
                    xn = s1.tile([128, HS], BF16, tag=f"xn_{s}")
                    nc.scalar.activation(out=xn[:], in_=x_nat[:],
                                         func=AF.Identity,
                                         scale=sig[:], bias=negmusig[:])
                    xnT = s1.tile([128, KT, 128], BF16, tag=f"xnT_{s}")
                    nc.sync.dma_start_transpose(out=xnT[:], in_=xn[:])
                    res[s] = xnT
                res["mask"] = mask_col
                return res

            def emit_B1(i, a):
                """proj k,v + elu(k) + vh_aug for tile i -> (phi_k, vh_aug)."""
                if a is None:
                    return None
                # k projection: kt outer / chunk inner (lhsT reuse)
                kh_ps = [proj_ps.tile([128, 512], F32, tag="proj",
                                      name="kh_ps")
                         for _ in range(2)]
                for kt in range(KT):
                    for c in range(2):
                        nc.tensor.matmul(
                            kh_ps[c][:], a["k"][:, kt, :],
                            w_sb["k"][:, kt, c * 512:(c + 1) * 512],
                            start=(kt == 0), stop=(kt == KT - 1))
                vh_ps = [proj_ps.tile([128, 512], F32, tag="proj",
                                      name="vh_ps")
                         for _ in range(2)]
                for kt in range(KT):
                    for c in range(2):
                        nc.tensor.matmul(
                            vh_ps[c][:], a["v"][:, kt, :],
                            w_sb["v"][:, kt, c * 512:(c + 1) * 512],
                            start=(kt == 0), stop=(kt == KT - 1))
                phi_k = s1.tile([128, HS], BF16, tag="phi_k")
                for c in range(2):
                    if c_bc["k"] is not None:
                        nc.vector.tensor_tensor(
                            out=kh_ps[c][:], in0=kh_ps[c][:],
                            in1=c_bc["k"][:, c * 512:(c + 1) * 512],
                            op=ALU.add)
                    _elu1(nc, s1, kh_ps[c], phi_k[:, c * 512:(c + 1) * 512],
                          a["mask"])
                vh_aug = s1.tile([128, NH, D + 1], BF16, tag="vh_aug")
                nc.vector.memset(vh_aug[:, :, D:D + 1], 1.0)
                for c in range(2):
                    if c_bc["v"] is not None:
                        nc.vector.tensor_tensor(
                            out=vh_ps[c][:], in0=vh_ps[c][:],
                            in1=c_bc["v"][:, c * 512:(c + 1) * 512],
                            op=ALU.add)
                    nc.scalar.activation(
                        out=vh_aug[:, c * 8:(c + 1) * 8, 0:D],
                        in_=vh_ps[c][:].rearrange("p (n d) -> p n d", d=D),
                        func=AF.Copy)
                return phi_k, vh_aug

            def emit_kv(i, b):
                if b is None:
                    return
                phi_k, vh_aug = b
                for n in range(NH):
                    beta, j, hs = n // 8, (n // 2) % 4, (n % 2) * 64
                    nc.tensor.matmul(
                        kv_ps[beta][hs:hs + 64, j, :],
                        phi_k[:, n * D:(n + 1) * D],
                        vh_aug[:, n, :],
                        start=(i == 0), stop=(i == NT - 1),
                        tile_position=(0, hs),
                        skip_group_check=True,
                    )

            for j in range(PF):
                emit_load1(j)
            a_cur = emit_A1(0)
            b_prev = None
            for i in range(NT):
                emit_load1(i + PF)
                a_next = emit_A1(i + 1)
                b_cur = emit_B1(i, a_cur)
                emit_kv(i - 1, b_prev)
                a_cur, b_prev = a_next, b_cur
            emit_kv(NT - 1, b_prev)

            nc.vector.tensor_copy(out=kv_sb[:, 0:4, :], in_=kv_ps[0][:])
            nc.vector.tensor_copy(out=kv_sb[:, 4:8, :], in_=kv_ps[1][:])

        # ---------------- all-reduce kv state within batch pairs ----------
        # kv2 holds the reduced state as 8 block-diagonal [128, 130] bf16
        # operands (head-pair 2m/2m+1), so the num/den matmul is a plain
        # K=128 matmul at base partition 0.
        kv2 = consts.tile([128, 8, 2 * (D + 1)], BF16, tag="kv2")
        nc.vector.memset(kv2[:], 0.0)
        if _skip_collective:
            nc.vector.tensor_copy(out=kv2[0:64, :, 0:D + 1],
                                  in_=kv_sb[0:64, :, :])
            nc.vector.tensor_copy(out=kv2[64:128, :, D + 1:2 * (D + 1)],
                                  in_=kv_sb[64:128, :, :])
        else:
            cc_in = dram_p.tile([128, 8, D + 1], F32)
            cc_out = dram_p.tile([128, 8, D + 1], F32)
            nc.gpsimd.dma_start(out=cc_in[:], in_=kv_sb[:])
            nc.gpsimd.collective_compute(
                "AllReduce", ALU.add, replica_groups=replica_groups,
                ins=[cc_in.opt()], outs=[cc_out.opt()],
            )
            nc.gpsimd.dma_start(out=kv2[0:64, :, 0:D + 1],
                                in_=cc_out[0:64, :, :])
            nc.gpsimd.dma_start(out=kv2[64:128, :, D + 1:2 * (D + 1)],
                                in_=cc_out[64:128, :, :])

        # ---------------- sweep 2: Q -> attn -> fc -> out ----------------
        with (
            tc.tile_pool(name="nd_ps", bufs=4, space="PSUM") as nd_psp,
            tc.tile_pool(name="s2", bufs=4) as s2,
            tc.tile_pool(name="q_ld", bufs=6) as q_ld,
            tc.tile_pool(name="stat2", bufs=8) as stat2,
        ):
            qloads = {}

            def emit_load2(i):
                if i >= NT:
                    return
                r0 = i * 128
                q_nat = q_ld.tile([128, HS], F32, tag="q_nat")
                nc.sync.dma_start(out=q_nat[:], in_=qx_d[r0:r0 + 128, :])
                qloads[i] = q_nat

            def emit_A2(i):
                """LN + xn + transpose of q tile i."""
                if i >= NT:
                    return None
                q_nat = qloads[i]
                sig, negmusig = _ln_stats(nc, stat2, q_nat)
                xn = s2.tile([128, HS], BF16, tag="xn_q")
                nc.scalar.activation(out=xn[:], in_=q_nat[:],
                                     func=AF.Identity,
                                     scale=sig[:], bias=negmusig[:])
                xnT = s2.tile([128, KT, 128], BF16, tag="xnT_q")
                nc.sync.dma_start_transpose(out=xnT[:], in_=xn[:])
                return xnT

            def emit_B2(i, xnT):
                """q projection + elu + phi_qT for tile i."""
                if xnT is None:
                    return None
                qh_ps = [proj_ps.tile([128, 512], F32, tag="proj",
                                      name="qh_ps")
                         for _ in range(2)]
                for kt in range(KT):
                    for c in range(2):
                        nc.tensor.matmul(
                            qh_ps[c][:], xnT[:, kt, :],
                            w_sb["q"][:, kt, c * 512:(c + 1) * 512],
                            start=(kt == 0), stop=(kt == KT - 1))
                phi_q = s2.tile([128, HS], BF16, tag="phi_q")
                for c in range(2):
                    if c_bc["q"] is not None:
                        nc.vector.tensor_tensor(
                            out=qh_ps[c][:], in0=qh_ps[c][:],
                            in1=c_bc["q"][:, c * 512:(c + 1) * 512],
                            op=ALU.add)
                    _elu1(nc, s2, qh_ps[c], phi_q[:, c * 512:(c + 1) * 512],
                          None)
                phi_qT = s2.tile([128, KT, 128], BF16, tag="phi_qT")
                nc.sync.dma_start_transpose(out=phi_qT[:], in_=phi_q[:])
                return phi_qT

            def emit_C2(i, phi_qT):
                """nd matmuls + den/rd + attn scaling + attnT for tile i."""
                if phi_qT is None:
                    return None
                nds = []
                for m in range(8):
                    if m % 2 == 0:
                        nd2 = nd_psp.tile([128, 2, 2 * (D + 1)], F32,
                                          tag="nd", name="nd",
                                          padded_shape=[128, 2, 256])
                        nds.append(nd2)
                    nc.tensor.matmul(
                        nd2[:, m % 2, :], phi_qT[:, m, :], kv2[:, m, :],
                        start=True, stop=True,
                    )
                den = stat2.tile([128, NH], F32, tag="den")
                for p in range(4):
                    nc.vector.tensor_copy(
                        out=den[:, 4 * p:4 * p + 4].rearrange(
                            "a (b c) -> a b c", b=2),
                        in_=nds[p][:, :, D::D + 1])
                rd = stat2.tile([128, NH], F32, tag="rd")
                nc.vector.tensor_scalar_add(out=rd[:], in0=den[:],
                                            scalar1=ATT_EPS)
                nc.vector.reciprocal(out=rd[:], in_=rd[:])
                attn = s2.tile([128, HS], BF16, tag="attn")
                for n in range(NH):
                    nd = nds[n // 4][:, (n // 2) % 2, :]
                    src = nd[:, (n % 2) * (D + 1):(n % 2) * (D + 1) + D]
                    if n % 2 == 0:
                        nc.scalar.activation(
                            out=attn[:, n * D:(n + 1) * D], in_=src,
                            func=AF.Copy, bias=0.0, scale=rd[:, n:n + 1])
                    else:
                        nc.vector.tensor_scalar_mul(
                            out=attn[:, n * D:(n + 1) * D], in0=src,
                            scalar1=rd[:, n:n + 1])
                attnT = s2.tile([128, KT, 128], BF16, tag="attnT")
                nc.sync.dma_start_transpose(out=attnT[:], in_=attn[:])
                return attnT

            def emit_D2(i, attnT):
                """fc + residual + store for tile i."""
                if attnT is None:
                    return
                q_nat = qloads.pop(i)
                fc_ps = [proj_ps.tile([128, 512], F32, tag="proj",
                                      name="fc_ps")
                         for _ in range(2)]
                for kt in range(KT):
                    for c in range(2):
                        nc.tensor.matmul(
                            fc_ps[c][:], attnT[:, kt, :],
                            w_sb["fc"][:, kt, c * 512:(c + 1) * 512],
                            start=(kt == 0), stop=(kt == KT - 1))
                out_sb = s2.tile([128, HS], F32, tag="out_sb")
                for c in range(2):
                    if fcb_bc is not None:
                        nc.vector.tensor_tensor(
                            out=fc_ps[c][:], in0=fc_ps[c][:],
                            in1=fcb_bc[:, c * 512:(c + 1) * 512], op=ALU.add)
                    nc.vector.tensor_tensor(
                        out=out_sb[:, c * 512:(c + 1) * 512], in0=fc_ps[c][:],
                        in1=q_nat[:, c * 512:(c + 1) * 512], op=ALU.add)
                r0 = i * 128
                nc.sync.dma_start(out=out_d[r0:r0 + 128, :], in_=out_sb[:])

            for j in range(PF):
                emit_load2(j)
            stA = emit_A2(0)
            stB = stC = None
            for i in range(NT + 2):
                emit_load2(i + PF)
                a_next = emit_A2(i + 1)
                b_next = emit_B2(i, stA) if i < NT else None
                c_next = emit_C2(i - 1, stB) if 1 <= i <= NT else None
                emit_D2(i - 2, stC)
                stA, stB, stC = a_next, b_next, c_next

    nc.compile()
    return nc


_BUILD_CACHE = {}


def _get_nc(flags, replica_groups):
    key = (flags, tuple(tuple(g) for g in replica_groups))
    if key not in _BUILD_CACHE:
        _BUILD_CACHE[key] = build(*flags, replica_groups)
    return _BUILD_CACHE[key]


def kernel(q, k, v, ln_q_g, ln_q_b, wq, bq, ln_k_g, ln_k_b, wk, bk,
           ln_v_g, ln_v_b, wv, bv, fc_w, fc_b, mask):
    q = np.ascontiguousarray(q, np.float32).reshape(B * S, HS)
    k = np.ascontiguousarray(k, np.float32).reshape(B * S, HS)
    v = np.ascontiguousarray(v, np.float32).reshape(B * S, HS)
    mask_f = np.ascontiguousarray(mask, np.float32).reshape(B * S, 1)
    wq = np.ascontiguousarray(wq, np.float32)
    wk = np.ascontiguousarray(wk, np.float32)
    wv = np.ascontiguousarray(wv, np.float32)
    fc_w = np.ascontiguousarray(fc_w, np.float32)

    g_trivial = all(np.all(x == 1.0) for x in (ln_q_g, ln_k_g, ln_v_g))
    c_trivial = all(np.all(x == 0.0) for x in
                    (ln_q_b, ln_k_b, ln_v_b, bq, bk, bv, fc_b))
    mask_trivial = bool(np.all(mask_f == 1.0))

    groups = [[0, 1], [2, 3], [4, 5], [6, 7]]
    nc = _get_nc((g_trivial, c_trivial, mask_trivial), groups)

    in_maps = []
    for c in range(NCORES):
        r0, r1 = c * TOK, (c + 1) * TOK
        m = {
            "qx": q[r0:r1], "kx": k[r0:r1], "vx": v[r0:r1],
            "wq": wq, "wk": wk, "wv": wv, "fcw": fc_w,
        }
        if not mask_trivial:
            m["maskx"] = mask_f[r0:r1]
        if not g_trivial:
            m.update({"g_q": np.asarray(ln_q_g, np.float32),
                      "g_k": np.asarray(ln_k_g, np.float32),
                      "g_v": np.asarray(ln_v_g, np.float32)})
        if not c_trivial:
            m.update({"b_q": np.asarray(ln_q_b, np.float32),
                      "b_k": np.asarray(ln_k_b, np.float32),
                      "b_v": np.asarray(ln_v_b, np.float32),
                      "pb_q": np.asarray(bq, np.float32),
                      "pb_k": np.asarray(bk, np.float32),
                      "pb_v": np.asarray(bv, np.float32),
                      "fcb": np.asarray(fc_b, np.float32)})
        in_maps.append(m)

    res = run_bass_kernel_spmd(nc, in_maps, list(range(NCORES)))
    out = np.concatenate([res.results[c]["out"] for c in range(NCORES)], 0)
    return out.reshape(B, S, HS).astype(np.float32)


# revision 4
# speedup vs baseline: 1.5371x; 1.0844x over previous
"""Trainium2 Bass kernel for nn_MultiHeadAttention_41936060678770.

LinBERT-style linear attention:
  qh/kh/vh = LN(x) @ W + b  (per-stream LN, 16 heads x 64 dim)
  phi = elu(.)+1 ;  phi_k masked
  kv = sum_s phi_k (x) vh ; z = sum_s phi_k
  attn = (phi_q @ kv) / (phi_q @ z + eps)
  out = q + attn @ fc_w + fc_b

Sharding: 8 cores, tokens split 8-ways over flattened (B*S); each pair of
cores (2c, 2c+1) holds one batch, so the [16,64,65] kv/z state is
all-reduced within core pairs; everything else is fully local.

v3 notes (745us baseline -> 526us v2 -> this):
  - host folds LN gain into W (xn*g@W = xn@(g*W)) and precomputes the
    combined bias c = b@W_eff + b_proj, and casts weights AND q/k/v to
    bf16 host-side: HBM input traffic drops 42MB -> 21MB and the gpsimd
    SWDGE cast path disappears (v2 lost ~45us at startup streaming f32
    weights);
  - sweep2 runs a 7-tile B-stage warmup before the first nd matmul so
    the in-order PE queue has ~30us of projection work queued while the
    kv AllReduce completes (v2 stalled 31us there);
  - xn transposes are launched from the scalar queue right after the
    ACT op that produces their input (no cross-engine wait blocking the
    load queue); loads/stores + phi_qT/attnT stay on sync;
  - PSUM: sweep1 uses separate kh/vh pools (3 bufs each) + 2 kv banks;
    sweep2 proj(4) + nd(4).
  - PE p-state: dense back-to-back matmul stream ramps the PE to 2.4GHz
    (it falls to 1.2GHz after idle gaps), so every stall costs double.
"""
import sys

sys.path.insert(0, "/opt/trn_rl_repo")

import ml_dtypes
import numpy as np

import concourse.bacc as bacc
import concourse.bass as bass
import concourse.tile as tile
import concourse.mybir as mybir
from concourse.bass_utils import run_bass_kernel_spmd

F32 = mybir.dt.float32
BF16 = mybir.dt.bfloat16
NP_BF16 = ml_dtypes.bfloat16
AF = mybir.ActivationFunctionType
ALU = mybir.AluOpType

B, S, HS = 4, 4096, 1024
NH, D = 16, 64
NCORES = 8
TOK = B * S // NCORES          # 2048 rows per core
NT = TOK // 128                # 16 token tiles
KT = HS // 128                 # 8 hidden tiles
LN_EPS = 1e-5
ATT_EPS = 1e-6
PF = 3                         # load prefetch depth (tiles)
WARM = 7                       # sweep2 B-stage warmup depth (hides AllReduce)


def _ln_stats(nc, stat_pool, x_nat):
    """bn stats + rsqrt(var+eps) via DVE newton. Returns (sig, negmusig)."""
    mv = stat_pool.tile([128, 2], F32, tag="mv")
    stats = stat_pool.tile([128, 2, 6], F32, tag="stats")
    nc.vector.bn_stats(out=stats[:, 0, :], in_=x_nat[:, 0:512])
    nc.vector.bn_stats(out=stats[:, 1, :], in_=x_nat[:, 512:1024])
    nc.vector.bn_aggr(out=mv[:], in_=stats[:])
    # rsig = rsqrt(var+eps) fully on DVE (quake seed + 2 Newton steps) so
    # ACT only ever runs one table set.
    veps = stat_pool.tile([128, 1], F32, tag="veps")
    nc.vector.tensor_scalar_add(out=veps[:], in0=mv[:, 1:2], scalar1=LN_EPS)
    seed = stat_pool.tile([128, 1], mybir.dt.int32, tag="seed")
    nc.vector.tensor_scalar(
        out=seed[:], in0=veps[:].bitcast(mybir.dt.int32),
        scalar1=1, scalar2=None, op0=ALU.arith_shift_right)
    nc.vector.tensor_scalar(
        out=seed[:], in0=seed[:], scalar1=-1, scalar2=0x5F3759DF,
        op0=ALU.mult, op1=ALU.add)
    y0 = seed[:].bitcast(F32)
    t_nr = stat_pool.tile([128, 1], F32, tag="t_nr")
    sig = stat_pool.tile([128, 1], F32, tag="sig")
    nc.vector.tensor_tensor(out=t_nr[:], in0=y0, in1=y0, op=ALU.mult)
    nc.vector.tensor_tensor(out=t_nr[:], in0=t_nr[:], in1=veps[:],
                            op=ALU.mult)
    nc.vector.tensor_scalar(out=t_nr[:], in0=t_nr[:], scalar1=-0.5,
                            scalar2=1.5, op0=ALU.mult, op1=ALU.add)
    nc.vector.tensor_tensor(out=sig[:], in0=y0, in1=t_nr[:], op=ALU.mult)
    nc.vector.tensor_tensor(out=t_nr[:], in0=sig[:], in1=sig[:], op=ALU.mult)
    nc.vector.tensor_tensor(out=t_nr[:], in0=t_nr[:], in1=veps[:],
                            op=ALU.mult)
    nc.vector.tensor_scalar(out=t_nr[:], in0=t_nr[:], scalar1=-0.5,
                            scalar2=1.5, op0=ALU.mult, op1=ALU.add)
    nc.vector.tensor_tensor(out=sig[:], in0=sig[:], in1=t_nr[:], op=ALU.mult)
    negmusig = stat_pool.tile([128, 1], F32, tag="negmusig")
    nc.vector.scalar_tensor_tensor(
        out=negmusig[:], in0=mv[:, 0:1], scalar=-1.0, in1=sig[:],
        op0=ALU.mult, op1=ALU.mult)
    return sig, negmusig


def _elu1(nc, pool, src_ps, out_ap, mask_col):
    """out = elu(src)+1 = max(src,0) + min(exp(src),1), optionally * mask.
    exp reads PSUM directly on ACT (values here never overflow exp)."""
    texp = pool.tile([128, 512], BF16, tag="texp")
    nc.scalar.activation(out=texp[:], in_=src_ps[:], func=AF.Exp)
    tmin = pool.tile([128, 512], BF16, tag="tmin")
    nc.vector.tensor_scalar_min(out=tmin[:], in0=texp[:], scalar1=1.0)
    if mask_col is None:
        nc.vector.scalar_tensor_tensor(
            out=out_ap, in0=src_ps[:], scalar=0.0, in1=tmin[:],
            op0=ALU.max, op1=ALU.add,
        )
    else:
        tphi = pool.tile([128, 512], F32, tag="tphi")
        nc.vector.scalar_tensor_tensor(
            out=tphi[:], in0=src_ps[:], scalar=0.0, in1=tmin[:],
            op0=ALU.max, op1=ALU.add,
        )
        nc.vector.tensor_scalar_mul(out=out_ap, in0=tphi[:], scalar1=mask_col)


def build(has_c: bool, has_mask: bool, replica_groups,
          _skip_collective=False):
    nc = bacc.Bacc(None)

    qx_d = nc.dram_tensor("qx", [TOK, HS], BF16, kind="ExternalInput")
    kx_d = nc.dram_tensor("kx", [TOK, HS], BF16, kind="ExternalInput")
    vx_d = nc.dram_tensor("vx", [TOK, HS], BF16, kind="ExternalInput")
    w_d = {s: nc.dram_tensor(f"w_{s}", [HS, HS], BF16, kind="ExternalInput")
           for s in ("q", "k", "v", "fc")}
    if has_mask:
        mask_d = nc.dram_tensor("maskx", [TOK, 1], F32, kind="ExternalInput")
    if has_c:
        c_d = {s: nc.dram_tensor(f"c_{s}", [HS], F32, kind="ExternalInput")
               for s in ("q", "k", "v", "fc")}

    out_d = nc.dram_tensor("out", [TOK, HS], F32, kind="ExternalOutput")

    from contextlib import ExitStack
    with tile.TileContext(nc) as tc, ExitStack() as ctx:
        wpool = ctx.enter_context(tc.tile_pool(name="weights", bufs=1))
        consts = ctx.enter_context(tc.tile_pool(name="consts", bufs=1))
        dram_p = ctx.enter_context(
            tc.tile_pool(name="dram", bufs=1, space="DRAM"))

        # ---------------- weights (bf16 from host, HWDGE chunked) ---------
        w_sb = {}
        w_src = {}
        for s in ("q", "k", "v", "fc"):
            w_sb[s] = wpool.tile([128, KT, HS], BF16, tag=f"w_{s}",
                                 name=f"w_{s}")
            w_src[s] = w_d[s].rearrange("(kt p) n -> p kt n", p=128)
        # k/v chunks first so sweep1's first projections start early
        for kt in range(KT):
            nc.sync.dma_start(out=w_sb["k"][:, kt, :],
                              in_=w_src["k"][:, kt, :])
            nc.sync.dma_start(out=w_sb["v"][:, kt, :],
                              in_=w_src["v"][:, kt, :])
        nc.sync.dma_start(out=w_sb["q"][:], in_=w_src["q"])
        nc.sync.dma_start(out=w_sb["fc"][:], in_=w_src["fc"])

        c_bc = {"q": None, "k": None, "v": None, "fc": None}
        if has_c:
            for s in ("q", "k", "v", "fc"):
                crow = consts.tile([1, HS], F32, tag=f"crow_{s}")
                nc.sync.dma_start(out=crow[:], in_=c_d[s][None, :])
                c_bc[s] = consts.tile([128, HS], F32, tag=f"cbc_{s}",
                                      name=f"cbc_{s}")
                nc.gpsimd.partition_broadcast(c_bc[s][:], crow[:])

        # ---------------- sweep 1: K/V + kv state ----------------
        kv_sb = consts.tile([128, 8, D + 1], F32, tag="kv_sb")
        with (
            tc.tile_pool(name="kv_ps", bufs=1, space="PSUM") as kv_psp,
            tc.tile_pool(name="kh_ps", bufs=3, space="PSUM") as kh_psp,
            tc.tile_pool(name="vh_ps", bufs=3, space="PSUM") as vh_psp,
            tc.tile_pool(name="s1", bufs=4) as s1,
            tc.tile_pool(name="ld1", bufs=5) as ld1,
            tc.tile_pool(name="stat1", bufs=8) as stat1,
        ):
            kv_ps = [kv_psp.tile([128, 4, D + 1], F32, tag=f"kv{b}",
                                 name=f"kv{b}", padded_shape=[128, 4, 128])
                     for b in range(2)]

            loads = {}

            def emit_load1(i):
                if i >= NT:
                    return
                r0 = i * 128
                k_nat = ld1.tile([128, HS], BF16, tag="k_nat")
                nc.sync.dma_start(out=k_nat[:], in_=kx_d[r0:r0 + 128, :])
                v_nat = ld1.tile([128, HS], BF16, tag="v_nat")
                nc.sync.dma_start(out=v_nat[:], in_=vx_d[r0:r0 + 128, :])
                mask_col = None
                if has_mask:
                    mcol = stat1.tile([128, 1], F32, tag="mcol")
                    nc.sync.dma_start(out=mcol[:], in_=mask_d[r0:r0 + 128, :])
                    mask_col = mcol[:]
                loads[i] = (k_nat, v_nat, mask_col)

            def emit_A1(i):
                """LN + xn + transpose for k and v of tile i."""
                if i >= NT:
                    return None
                k_nat, v_nat, mask_col = loads.pop(i)
                res = {}
                for s, x_nat in (("k", k_nat), ("v", v_nat)):
                    sig, negmusig = _ln_stats(nc, stat1, x_nat)
                    xn = s1.tile([128, HS], BF16, tag=f"xn_{s}")
                    nc.scalar.activation(out=xn[:], in_=x_nat[:],
                                         func=AF.Identity,
                                         scale=sig[:], bias=negmusig[:])
                    xnT = s1.tile([128, KT, 128], BF16, tag=f"xnT_{s}")
                    # launched from the scalar queue right after its producer
                    nc.scalar.dma_start_transpose(out=xnT[:], in_=xn[:])
                    res[s] = xnT
                res["mask"] = mask_col
                return res

            def emit_B1(i, a):
                """proj k,v + elu(k) + vh_aug for tile i -> (phi_k, vh_aug)."""
                if a is None:
                    return None
                kh_ps = [kh_psp.tile([128, 512], F32, tag="proj",
                                     name="kh_ps")
                         for _ in range(2)]
                for kt in range(KT):
                    for c in range(2):
                        nc.tensor.matmul(
                            kh_ps[c][:], a["k"][:, kt, :],
                            w_sb["k"][:, kt, c * 512:(c + 1) * 512],
                            start=(kt == 0), stop=(kt == KT - 1))
                vh_ps = [vh_psp.tile([128, 512], F32, tag="proj",
                                     name="vh_ps")
                         for _ in range(2)]
                for kt in range(KT):
                    for c in range(2):
                        nc.tensor.matmul(
                            vh_ps[c][:], a["v"][:, kt, :],
                            w_sb["v"][:, kt, c * 512:(c + 1) * 512],
                            start=(kt == 0), stop=(kt == KT - 1))
                phi_k = s1.tile([128, HS], BF16, tag="phi_k")
                for c in range(2):
                    if c_bc["k"] is not None:
                        nc.vector.tensor_tensor(
                            out=kh_ps[c][:], in0=kh_ps[c][:],
                            in1=c_bc["k"][:, c * 512:(c + 1) * 512],
                            op=ALU.add)
                    _elu1(nc, s1, kh_ps[c], phi_k[:, c * 512:(c + 1) * 512],
                          a["mask"])
                vh_aug = s1.tile([128, NH, D + 1], BF16, tag="vh_aug")
                nc.vector.memset(vh_aug[:, :, D:D + 1], 1.0)
                for c in range(2):
                    if c_bc["v"] is not None:
                        nc.vector.tensor_tensor(
                            out=vh_ps[c][:], in0=vh_ps[c][:],
                            in1=c_bc["v"][:, c * 512:(c + 1) * 512],
                            op=ALU.add)
                    nc.scalar.activation(
                        out=vh_aug[:, c * 8:(c + 1) * 8, 0:D],
                        in_=vh_ps[c][:].rearrange("p (n d) -> p n d", d=D),
                        func=AF.Copy)
                return phi_k, vh_aug

            def emit_kv(i, b):
                if b is None:
                    return
                phi_k, vh_aug = b
                for n in range(NH):
                    beta, j, hs = n // 8, (n // 2) % 4, (n % 2) * 64
                    nc.tensor.matmul(
                        kv_ps[beta][hs:hs + 64, j, :],
                        phi_k[:, n * D:(n + 1) * D],
                        vh_aug[:, n, :],
                        start=(i == 0), stop=(i == NT - 1),
                        tile_position=(0, hs),
                        skip_group_check=True,
                    )

            for j in range(PF):
                emit_load1(j)
            a_cur = emit_A1(0)
            b_prev = None
            for i in range(NT):
                emit_load1(i + PF)
                a_next = emit_A1(i + 1)
                b_cur = emit_B1(i, a_cur)
                emit_kv(i - 1, b_prev)
                a_cur, b_prev = a_next, b_cur
            emit_kv(NT - 1, b_prev)

            nc.vector.tensor_copy(out=kv_sb[:, 0:4, :], in_=kv_ps[0][:])
            nc.vector.tensor_copy(out=kv_sb[:, 4:8, :], in_=kv_ps[1][:])

        # ---------------- all-reduce kv state within batch pairs ----------
        # kv2 holds the reduced state as 8 block-diagonal [128, 130] bf16
        # operands (head-pair 2m/2m+1), so the num/den matmul is a plain
        # K=128 matmul at base partition 0.
        kv2 = consts.tile([128, 8, 2 * (D + 1)], BF16, tag="kv2")
        nc.vector.memset(kv2[:], 0.0)
        if _skip_collective:
            nc.vector.tensor_copy(out=kv2[0:64, :, 0:D + 1],
                                  in_=kv_sb[0:64, :, :])
            nc.vector.tensor_copy(out=kv2[64:128, :, D + 1:2 * (D + 1)],
                                  in_=kv_sb[64:128, :, :])
        else:
            cc_in = dram_p.tile([128, 8, D + 1], F32)
            cc_out = dram_p.tile([128, 8, D + 1], F32)
            nc.gpsimd.dma_start(out=cc_in[:], in_=kv_sb[:])
            nc.gpsimd.collective_compute(
                "AllReduce", ALU.add, replica_groups=replica_groups,
                ins=[cc_in.opt()], outs=[cc_out.opt()],
            )
            nc.gpsimd.dma_start(out=kv2[0:64, :, 0:D + 1],
                                in_=cc_out[0:64, :, :])
            nc.gpsimd.dma_start(out=kv2[64:128, :, D + 1:2 * (D + 1)],
                                in_=cc_out[64:128, :, :])

        # ---------------- sweep 2: Q -> attn -> fc -> out ----------------
        with (
            tc.tile_pool(name="proj_ps", bufs=4, space="PSUM") as proj_ps,
            tc.tile_pool(name="nd_ps", bufs=4, space="PSUM") as nd_psp,
            tc.tile_pool(name="s2", bufs=4) as s2,
            tc.tile_pool(name="pqt", bufs=WARM + 1) as pqt,
            tc.tile_pool(name="q_ld", bufs=WARM + PF + 2) as q_ld,
            tc.tile_pool(name="stat2", bufs=8) as stat2,
        ):
            qloads = {}
            pqts = {}
            attnTs = {}

            def emit_load2(i):
                if i >= NT:
                    return
                r0 = i * 128
                q_nat = q_ld.tile([128, HS], BF16, tag="q_nat")
                nc.sync.dma_start(out=q_nat[:], in_=qx_d[r0:r0 + 128, :])
                qloads[i] = q_nat

            def emit_A2(i):
                """LN + xn + transpose of q tile i."""
                if i >= NT:
                    return None
                q_nat = qloads[i]
                sig, negmusig = _ln_stats(nc, stat2, q_nat)
                xn = s2.tile([128, HS], BF16, tag="xn_q")
                nc.scalar.activation(out=xn[:], in_=q_nat[:],
                                     func=AF.Identity,
                                     scale=sig[:], bias=negmusig[:])
                xnT = s2.tile([128, KT, 128], BF16, tag="xnT_q")
                nc.scalar.dma_start_transpose(out=xnT[:], in_=xn[:])
                return xnT

            def emit_B2(i, xnT):
                """q projection + elu + phi_qT for tile i."""
                if xnT is None:
                    return
                qh_ps = [proj_ps.tile([128, 512], F32, tag="proj",
                                      name="qh_ps")
                         for _ in range(2)]
                for kt in range(KT):
                    for c in range(2):
                        nc.tensor.matmul(
                            qh_ps[c][:], xnT[:, kt, :],
                            w_sb["q"][:, kt, c * 512:(c + 1) * 512],
                            start=(kt == 0), stop=(kt == KT - 1))
                phi_q = s2.tile([128, HS], BF16, tag="phi_q")
                for c in range(2):
                    if c_bc["q"] is not None:
                        nc.vector.tensor_tensor(
                            out=qh_ps[c][:], in0=qh_ps[c][:],
                            in1=c_bc["q"][:, c * 512:(c + 1) * 512],
                            op=ALU.add)
                    _elu1(nc, s2, qh_ps[c], phi_q[:, c * 512:(c + 1) * 512],
                          None)
                phi_qT = pqt.tile([128, KT, 128], BF16, tag="phi_qT")
                nc.sync.dma_start_transpose(out=phi_qT[:], in_=phi_q[:])
                pqts[i] = phi_qT

            def emit_C2(i):
                """nd matmuls + den/rd + attn scaling + attnT for tile i."""
                if not (0 <= i < NT):
                    return
                phi_qT = pqts.pop(i)
                nds = []
                for m in range(8):
                    if m % 2 == 0:
                        nd2 = nd_psp.tile([128, 2, 2 * (D + 1)], F32,
                                          tag="nd", name="nd",
                                          padded_shape=[128, 2, 256])
                        nds.append(nd2)
                    nc.tensor.matmul(
                        nd2[:, m % 2, :], phi_qT[:, m, :], kv2[:, m, :],
                        start=True, stop=True,
                    )
                den = stat2.tile([128, NH], F32, tag="den")
                for p in range(4):
                    nc.vector.tensor_copy(
                        out=den[:, 4 * p:4 * p + 4].rearrange(
                            "a (b c) -> a b c", b=2),
                        in_=nds[p][:, :, D::D + 1])
                rd = stat2.tile([128, NH], F32, tag="rd")
                nc.vector.tensor_scalar_add(out=rd[:], in0=den[:],
                                            scalar1=ATT_EPS)
                nc.vector.reciprocal(out=rd[:], in_=rd[:])
                attn = s2.tile([128, HS], BF16, tag="attn")
                for n in range(NH):
                    nd = nds[n // 4][:, (n // 2) % 2, :]
                    src = nd[:, (n % 2) * (D + 1):(n % 2) * (D + 1) + D]
                    if n % 2 == 0:
                        nc.scalar.activation(
                            out=attn[:, n * D:(n + 1) * D], in_=src,
                            func=AF.Copy, bias=0.0, scale=rd[:, n:n + 1])
                    else:
                        nc.vector.tensor_scalar_mul(
                            out=attn[:, n * D:(n + 1) * D], in0=src,
                            scalar1=rd[:, n:n + 1])
                attnT = s2.tile([128, KT, 128], BF16, tag="attnT")
                nc.sync.dma_start_transpose(out=attnT[:], in_=attn[:])
                attnTs[i] = attnT

            def emit_D2(i):
                """fc + residual + store for tile i."""
                if not (0 <= i < NT):
                    return
                attnT = attnTs.pop(i)
                q_nat = qloads.pop(i)
                fc_ps = [proj_ps.tile([128, 512], F32, tag="proj",
                                      name="fc_ps")
                         for _ in range(2)]
                for kt in range(KT):
                    for c in range(2):
                        nc.tensor.matmul(
                            fc_ps[c][:], attnT[:, kt, :],
                            w_sb["fc"][:, kt, c * 512:(c + 1) * 512],
                            start=(kt == 0), stop=(kt == KT - 1))
                out_sb = s2.tile([128, HS], F32, tag="out_sb")
                for c in range(2):
                    if c_bc["fc"] is not None:
                        nc.vector.tensor_tensor(
                            out=fc_ps[c][:], in0=fc_ps[c][:],
                            in1=c_bc["fc"][:, c * 512:(c + 1) * 512],
                            op=ALU.add)
                    nc.vector.tensor_tensor(
                        out=out_sb[:, c * 512:(c + 1) * 512], in0=fc_ps[c][:],
                        in1=q_nat[:, c * 512:(c + 1) * 512], op=ALU.add)
                r0 = i * 128
                nc.sync.dma_start(out=out_d[r0:r0 + 128, :], in_=out_sb[:])

            for j in range(PF):
                emit_load2(j)
            stA = emit_A2(0)
            for i in range(NT + WARM + 1):
                emit_load2(i + PF)
                a_next = emit_A2(i + 1)
                emit_B2(i, stA)
                stA = a_next
                emit_C2(i - WARM)
                emit_D2(i - WARM - 1)

    nc.compile()
    return nc


_BUILD_CACHE = {}


def _get_nc(flags, replica_groups):
    key = (flags, tuple(tuple(g) for g in replica_groups))
    if key not in _BUILD_CACHE:
        _BUILD_CACHE[key] = build(*flags, replica_groups)
    return _BUILD_CACHE[key]


def host_prep(q, k, v, ln_q_g, ln_q_b, wq, bq, ln_k_g, ln_k_b, wk, bk,
              ln_v_g, ln_v_b, wv, bv, fc_w, fc_b, mask):
    """Fold LN gains into W, combine biases, cast to bf16. Returns
    (flags, in_maps, groups)."""
    q = np.ascontiguousarray(q, np.float32).reshape(B * S, HS)
    k = np.ascontiguousarray(k, np.float32).reshape(B * S, HS)
    v = np.ascontiguousarray(v, np.float32).reshape(B * S, HS)
    mask_f = np.ascontiguousarray(mask, np.float32).reshape(B * S, 1)

    w_eff = {}
    c_eff = {}
    for s, g, b, w, pb in (("q", ln_q_g, ln_q_b, wq, bq),
                           ("k", ln_k_g, ln_k_b, wk, bk),
                           ("v", ln_v_g, ln_v_b, wv, bv)):
        g = np.asarray(g, np.float32)
        b = np.asarray(b, np.float32)
        w = np.asarray(w, np.float32)
        pb = np.asarray(pb, np.float32)
        we = w * g[:, None] if not np.all(g == 1.0) else w
        w_eff[s] = np.ascontiguousarray(we.astype(NP_BF16))
        c_eff[s] = (b @ we + pb).astype(np.float32)
    w_eff["fc"] = np.ascontiguousarray(
        np.asarray(fc_w, np.float32).astype(NP_BF16))
    c_eff["fc"] = np.asarray(fc_b, np.float32)

    has_c = any(np.any(c != 0.0) for c in c_eff.values())
    has_mask = not bool(np.all(mask_f == 1.0))

    qb = np.ascontiguousarray(q.astype(NP_BF16))
    kb = np.ascontiguousarray(k.astype(NP_BF16))
    vb = np.ascontiguousarray(v.astype(NP_BF16))

    in_maps = []
    for c in range(NCORES):
        r0, r1 = c * TOK, (c + 1) * TOK
        m = {
            "qx": qb[r0:r1], "kx": kb[r0:r1], "vx": vb[r0:r1],
            "w_q": w_eff["q"], "w_k": w_eff["k"], "w_v": w_eff["v"],
            "w_fc": w_eff["fc"],
        }
        if has_mask:
            m["maskx"] = mask_f[r0:r1]
        if has_c:
            for s in ("q", "k", "v", "fc"):
                m[f"c_{s}"] = c_eff[s]
        in_maps.append(m)

    groups = [[0, 1], [2, 3], [4, 5], [6, 7]]
    return (has_c, has_mask), in_maps, groups


def kernel(**inputs):
    flags, in_maps, groups = host_prep(**inputs)
    nc = _get_nc(flags, groups)
    res = run_bass_kernel_spmd(nc, in_maps, list(range(NCORES)))
    out = np.concatenate([res.results[c]["out"] for c in range(NCORES)], 0)
    return out.reshape(B, S, HS).astype(np.float32)


# revision 10
# speedup vs baseline: 1.5896x; 1.0341x over previous
"""Trainium2 Bass kernel for nn_MultiHeadAttention_41936060678770.

LinBERT-style linear attention:
  qh/kh/vh = LN(x) @ W + b  (per-stream LN, 16 heads x 64 dim)
  phi = elu(.)+1 ;  phi_k masked
  kv = sum_s phi_k (x) vh ; z = sum_s phi_k
  attn = (phi_q @ kv) / (phi_q @ z + eps)
  out = q + attn @ fc_w + fc_b

Sharding: 8 cores, tokens split 8-ways over flattened (B*S); each pair of
cores (2c, 2c+1) holds one batch, so the [16,64,65] kv/z state is
all-reduced within core pairs; everything else is fully local.

v4 (745us baseline -> 526 -> 485 -> this). Trace-driven changes:
  - host folds LN gain into W, precomputes combined bias, casts weights
    and q/k/v to bf16 (HBM input traffic 42MB -> 21MB);
  - weights load via the otherwise-idle gpsimd queue (one DMA each) so
    the sync queue's ~0.7us/launch budget goes to activation tiles; the
    485us trace lost 53us at startup to weight/load launch contention;
  - all 16 q tiles load + LN + transpose during sweep1's tail (their
    queues have slack), so sweep2's projections start the moment
    sweep1's PE work drains and the kv AllReduce is fully hidden behind
    a 7-tile projection warmup;
  - vh_aug copies and the residual adds run on gpsimd (Pool), exp/xn/
    half the attn scales on ACT, stats/newton/elu-min/stt/den on DVE:
    every engine stays under the PE's ~9.4us/tile;
  - PE p-state: dense back-to-back matmuls ramp the PE to 2.4GHz; idle
    gaps drop it to 1.2GHz, so the whole design minimizes PE stalls.
"""
import sys

sys.path.insert(0, "/opt/trn_rl_repo")

import ml_dtypes
import numpy as np

import concourse.bacc as bacc
import concourse.bass as bass
import concourse.tile as tile
import concourse.mybir as mybir
from concourse.bass_utils import run_bass_kernel_spmd

F32 = mybir.dt.float32
BF16 = mybir.dt.bfloat16
NP_BF16 = ml_dtypes.bfloat16
AF = mybir.ActivationFunctionType
ALU = mybir.AluOpType

B, S, HS = 4, 4096, 1024
NH, D = 16, 64
NCORES = 8
TOK = B * S // NCORES          # 2048 rows per core
NT = TOK // 128                # 16 token tiles
KT = HS // 128                 # 8 hidden tiles
LN_EPS = 1e-5
ATT_EPS = 1e-6
PF = 3                         # sweep1 k/v load prefetch depth (tiles)
WARM = 7                       # sweep2 projection warmup depth (hides CC)


def _ln_stats(nc, stat_pool, x_nat):
    """bn stats + rsqrt(var+eps) via DVE newton. Returns (sig, negmusig)."""
    mv = stat_pool.tile([128, 2], F32, tag="mv")
    stats = stat_pool.tile([128, 2, 6], F32, tag="stats")
    nc.vector.bn_stats(out=stats[:, 0, :], in_=x_nat[:, 0:512])
    nc.vector.bn_stats(out=stats[:, 1, :], in_=x_nat[:, 512:1024])
    nc.vector.bn_aggr(out=mv[:], in_=stats[:])
    # rsig = rsqrt(var+eps) fully on DVE (quake seed + 2 Newton steps) so
    # ACT only ever runs one table set.
    veps = stat_pool.tile([128, 1], F32, tag="veps")
    nc.vector.tensor_scalar_add(out=veps[:], in0=mv[:, 1:2], scalar1=LN_EPS)
    seed = stat_pool.tile([128, 1], mybir.dt.int32, tag="seed")
    nc.vector.tensor_scalar(
        out=seed[:], in0=veps[:].bitcast(mybir.dt.int32),
        scalar1=1, scalar2=None, op0=ALU.arith_shift_right)
    nc.vector.tensor_scalar(
        out=seed[:], in0=seed[:], scalar1=-1, scalar2=0x5F3759DF,
        op0=ALU.mult, op1=ALU.add)
    y0 = seed[:].bitcast(F32)
    t_nr = stat_pool.tile([128, 1], F32, tag="t_nr")
    sig = stat_pool.tile([128, 1], F32, tag="sig")
    nc.vector.tensor_tensor(out=t_nr[:], in0=y0, in1=y0, op=ALU.mult)
    nc.vector.tensor_tensor(out=t_nr[:], in0=t_nr[:], in1=veps[:],
                            op=ALU.mult)
    nc.vector.tensor_scalar(out=t_nr[:], in0=t_nr[:], scalar1=-0.5,
                            scalar2=1.5, op0=ALU.mult, op1=ALU.add)
    nc.vector.tensor_tensor(out=sig[:], in0=y0, in1=t_nr[:], op=ALU.mult)
    nc.vector.tensor_tensor(out=t_nr[:], in0=sig[:], in1=sig[:], op=ALU.mult)
    nc.vector.tensor_tensor(out=t_nr[:], in0=t_nr[:], in1=veps[:],
                            op=ALU.mult)
    nc.vector.tensor_scalar(out=t_nr[:], in0=t_nr[:], scalar1=-0.5,
                            scalar2=1.5, op0=ALU.mult, op1=ALU.add)
    nc.vector.tensor_tensor(out=sig[:], in0=sig[:], in1=t_nr[:], op=ALU.mult)
    negmusig = stat_pool.tile([128, 1], F32, tag="negmusig")
    nc.vector.scalar_tensor_tensor(
        out=negmusig[:], in0=mv[:, 0:1], scalar=-1.0, in1=sig[:],
        op0=ALU.mult, op1=ALU.mult)
    return sig, negmusig


def _elu1(nc, pool, src_ps, out_ap, mask_col):
    """out = elu(src)+1 = max(src,0) + min(exp(src),1), optionally * mask.
    exp reads PSUM directly on ACT (values here never overflow exp)."""
    texp = pool.tile([128, 512], BF16, tag="texp")
    nc.scalar.activation(out=texp[:], in_=src_ps[:], func=AF.Exp)
    tmin = pool.tile([128, 512], BF16, tag="tmin")
    nc.vector.tensor_scalar_min(out=tmin[:], in0=texp[:], scalar1=1.0)
    if mask_col is None:
        nc.vector.scalar_tensor_tensor(
            out=out_ap, in0=src_ps[:], scalar=0.0, in1=tmin[:],
            op0=ALU.max, op1=ALU.add,
        )
    else:
        tphi = pool.tile([128, 512], F32, tag="tphi")
        nc.vector.scalar_tensor_tensor(
            out=tphi[:], in0=src_ps[:], scalar=0.0, in1=tmin[:],
            op0=ALU.max, op1=ALU.add,
        )
        nc.vector.tensor_scalar_mul(out=out_ap, in0=tphi[:], scalar1=mask_col)


def build(has_c: bool, has_mask: bool, replica_groups,
          _skip_collective=False):
    nc = bacc.Bacc(None)

    qx_d = nc.dram_tensor("qx", [TOK, HS], BF16, kind="ExternalInput")
    kx_d = nc.dram_tensor("kx", [TOK, HS], BF16, kind="ExternalInput")
    vx_d = nc.dram_tensor("vx", [TOK, HS], BF16, kind="ExternalInput")
    w_d = {s: nc.dram_tensor(f"w_{s}", [HS, HS], BF16, kind="ExternalInput")
           for s in ("q", "k", "v", "fc")}
    if has_mask:
        mask_d = nc.dram_tensor("maskx", [TOK, 1], F32, kind="ExternalInput")
    if has_c:
        c_d = {s: nc.dram_tensor(f"c_{s}", [HS], F32, kind="ExternalInput")
               for s in ("q", "k", "v", "fc")}

    out_d = nc.dram_tensor("out", [TOK, HS], F32, kind="ExternalOutput")

    from contextlib import ExitStack
    with tile.TileContext(nc) as tc, ExitStack() as ctx:
        wpool = ctx.enter_context(tc.tile_pool(name="weights", bufs=1))
        consts = ctx.enter_context(tc.tile_pool(name="consts", bufs=1))
        dram_p = ctx.enter_context(
            tc.tile_pool(name="dram", bufs=1, space="DRAM"))
        # q-side pools live across both sweeps (q prep happens in sweep1)
        q_ld = ctx.enter_context(tc.tile_pool(name="q_ld", bufs=NT))
        qw_xn = ctx.enter_context(tc.tile_pool(name="qw_xn", bufs=3))
        qw_xnT = ctx.enter_context(tc.tile_pool(name="qw_xnT", bufs=WARM + 2))
        stat2 = ctx.enter_context(tc.tile_pool(name="stat2", bufs=8))

        # ---------------- weights (bf16 from host, gpsimd queue) ----------
        w_sb = {}
        for s in ("k", "v", "q", "fc"):
            w_sb[s] = wpool.tile([128, KT, HS], BF16, tag=f"w_{s}",
                                 name=f"w_{s}")
            nc.gpsimd.dma_start(
                out=w_sb[s][:],
                in_=w_d[s].rearrange("(kt p) n -> p kt n", p=128))

        c_bc = {"q": None, "k": None, "v": None, "fc": None}
        if has_c:
            for s in ("q", "k", "v", "fc"):
                crow = consts.tile([1, HS], F32, tag=f"crow_{s}")
                nc.sync.dma_start(out=crow[:], in_=c_d[s][None, :])
                c_bc[s] = consts.tile([128, HS], F32, tag=f"cbc_{s}",
                                      name=f"cbc_{s}")
                nc.gpsimd.partition_broadcast(c_bc[s][:], crow[:])

        qloads = {}
        qxnTs = {}

        def emit_load2(i):
            if not (0 <= i < NT):
                return
            r0 = i * 128
            q_nat = q_ld.tile([128, HS], BF16, tag="q_nat")
            nc.sync.dma_start(out=q_nat[:], in_=qx_d[r0:r0 + 128, :])
            qloads[i] = q_nat

        def emit_A2(i):
            """LN + xn + transpose of q tile i (runs during sweep1 tail)."""
            if not (0 <= i < NT):
                return
            q_nat = qloads[i]
            sig, negmusig = _ln_stats(nc, stat2, q_nat)
            xn = qw_xn.tile([128, HS], BF16, tag="xn_q")
            nc.scalar.activation(out=xn[:], in_=q_nat[:], func=AF.Identity,
                                 scale=sig[:], bias=negmusig[:])
            xnT = qw_xnT.tile([128, KT, 128], BF16, tag="xnT_q")
            nc.sync.dma_start_transpose(out=xnT[:], in_=xn[:])
            qxnTs[i] = xnT

        # ---------------- sweep 1: K/V + kv state ----------------
        kv_sb = consts.tile([128, 8, D + 1], F32, tag="kv_sb")
        with (
            tc.tile_pool(name="kv_ps", bufs=1, space="PSUM") as kv_psp,
            tc.tile_pool(name="kh_ps", bufs=3, space="PSUM") as kh_psp,
            tc.tile_pool(name="vh_ps", bufs=3, space="PSUM") as vh_psp,
            tc.tile_pool(name="s1", bufs=3) as s1,
            tc.tile_pool(name="ld1", bufs=6) as ld1,
            tc.tile_pool(name="stat1", bufs=8) as stat1,
        ):
            kv_ps = [kv_psp.tile([128, 4, D + 1], F32, tag=f"kv{b}",
                                 name=f"kv{b}", padded_shape=[128, 4, 128])
                     for b in range(2)]

            loads = {}

            def emit_load1(i):
                if i >= NT:
                    return
                r0 = i * 128
                k_nat = ld1.tile([128, HS], BF16, tag="k_nat")
                nc.sync.dma_start(out=k_nat[:], in_=kx_d[r0:r0 + 128, :])
                v_nat = ld1.tile([128, HS], BF16, tag="v_nat")
                nc.sync.dma_start(out=v_nat[:], in_=vx_d[r0:r0 + 128, :])
                mask_col = None
                if has_mask:
                    mcol = stat1.tile([128, 1], F32, tag="mcol")
                    nc.sync.dma_start(out=mcol[:], in_=mask_d[r0:r0 + 128, :])
                    mask_col = mcol[:]
                loads[i] = (k_nat, v_nat, mask_col)

            def emit_A1(i):
                """LN + xn + transpose for k and v of tile i."""
                if i >= NT:
                    return None
                k_nat, v_nat, mask_col = loads.pop(i)
                res = {}
                for s, x_nat in (("k", k_nat), ("v", v_nat)):
                    sig, negmusig = _ln_stats(nc, stat1, x_nat)
                    xn = s1.tile([128, HS], BF16, tag=f"xn_{s}")
                    nc.scalar.activation(out=xn[:], in_=x_nat[:],
                                         func=AF.Identity,
                                         scale=sig[:], bias=negmusig[:])
                    xnT = s1.tile([128, KT, 128], BF16, tag=f"xnT_{s}")
                    nc.sync.dma_start_transpose(out=xnT[:], in_=xn[:])
                    res[s] = xnT
                res["mask"] = mask_col
                return res

            def emit_B1(i, a):
                """proj k,v + elu(k) + vh_aug for tile i -> (phi_k, vh_aug)."""
                if a is None:
                    return None
                kh_ps = [kh_psp.tile([128, 512], F32, tag="proj",
                                     name="kh_ps")
                         for _ in range(2)]
                for kt in range(KT):
                    for c in range(2):
                        nc.tensor.matmul(
                            kh_ps[c][:], a["k"][:, kt, :],
                            w_sb["k"][:, kt, c * 512:(c + 1) * 512],
                            start=(kt == 0), stop=(kt == KT - 1))
                vh_ps = [vh_psp.tile([128, 512], F32, tag="proj",
                                     name="vh_ps")
                         for _ in range(2)]
                for kt in range(KT):
                    for c in range(2):
                        nc.tensor.matmul(
                            vh_ps[c][:], a["v"][:, kt, :],
                            w_sb["v"][:, kt, c * 512:(c + 1) * 512],
                            start=(kt == 0), stop=(kt == KT - 1))
                phi_k = s1.tile([128, HS], BF16, tag="phi_k")
                for c in range(2):
                    if c_bc["k"] is not None:
                        nc.vector.tensor_tensor(
                            out=kh_ps[c][:], in0=kh_ps[c][:],
                            in1=c_bc["k"][:, c * 512:(c + 1) * 512],
                            op=ALU.add)
                    _elu1(nc, s1, kh_ps[c], phi_k[:, c * 512:(c + 1) * 512],
                          a["mask"])
                vh_aug = s1.tile([128, NH, D + 1], BF16, tag="vh_aug")
                nc.gpsimd.memset(vh_aug[:, :, D:D + 1], 1.0)
                for c in range(2):
                    if c_bc["v"] is not None:
                        nc.vector.tensor_tensor(
                            out=vh_ps[c][:], in0=vh_ps[c][:],
                            in1=c_bc["v"][:, c * 512:(c + 1) * 512],
                            op=ALU.add)
                    nc.scalar.activation(
                        out=vh_aug[:, c * 8:(c + 1) * 8, 0:D],
                        in_=vh_ps[c][:].rearrange("p (n d) -> p n d", d=D),
                        func=AF.Copy)
                return phi_k, vh_aug

            def emit_kv(i, b):
                if b is None:
                    return
                phi_k, vh_aug = b
                for n in range(NH):
                    beta, j, hs = n // 8, (n // 2) % 4, (n % 2) * 64
                    nc.tensor.matmul(
                        kv_ps[beta][hs:hs + 64, j, :],
                        phi_k[:, n * D:(n + 1) * D],
                        vh_aug[:, n, :],
                        start=(i == 0), stop=(i == NT - 1),
                        tile_position=(0, hs),
                        skip_group_check=True,
                    )

            for j in range(PF):
                emit_load1(j)
            a_cur = emit_A1(0)
            b_prev = None
            for i in range(NT):
                emit_load1(i + PF)
                # q prep rides sweep1's spare DVE/ACT/sync capacity:
                # all q loads early, LN+transpose for the first WARM tiles.
                emit_load2(i)
                if i >= NT - WARM:
                    emit_A2(i - (NT - WARM))
                a_next = emit_A1(i + 1)
                b_cur = emit_B1(i, a_cur)
                emit_kv(i - 1, b_prev)
                a_cur, b_prev = a_next, b_cur
            emit_kv(NT - 1, b_prev)

            nc.vector.tensor_copy(out=kv_sb[:, 0:4, :], in_=kv_ps[0][:])
            nc.vector.tensor_copy(out=kv_sb[:, 4:8, :], in_=kv_ps[1][:])

        # ---------------- all-reduce kv state within batch pairs ----------
        # kv2 holds the reduced state as 8 block-diagonal [128, 130] bf16
        # operands (head-pair 2m/2m+1), so the num/den matmul is a plain
        # K=128 matmul at base partition 0.
        kv2 = consts.tile([128, 8, 2 * (D + 1)], BF16, tag="kv2")
        nc.vector.memset(kv2[:], 0.0)
        if _skip_collective:
            nc.vector.tensor_copy(out=kv2[0:64, :, 0:D + 1],
                                  in_=kv_sb[0:64, :, :])
            nc.vector.tensor_copy(out=kv2[64:128, :, D + 1:2 * (D + 1)],
                                  in_=kv_sb[64:128, :, :])
        else:
            cc_in = dram_p.tile([128, 8, D + 1], F32)
            cc_out = dram_p.tile([128, 8, D + 1], F32)
            nc.gpsimd.dma_start(out=cc_in[:], in_=kv_sb[:])
            nc.gpsimd.collective_compute(
                "AllReduce", ALU.add, replica_groups=replica_groups,
                ins=[cc_in.opt()], outs=[cc_out.opt()],
            )
            nc.gpsimd.dma_start(out=kv2[0:64, :, 0:D + 1],
                                in_=cc_out[0:64, :, :])
            nc.gpsimd.dma_start(out=kv2[64:128, :, D + 1:2 * (D + 1)],
                                in_=cc_out[64:128, :, :])

        # ---------------- sweep 2: Q -> attn -> fc -> out ----------------
        with (
            tc.tile_pool(name="proj_ps", bufs=4, space="PSUM") as proj_ps,
            tc.tile_pool(name="nd_ps", bufs=4, space="PSUM") as nd_psp,
            tc.tile_pool(name="s2", bufs=4) as s2,
            tc.tile_pool(name="pqt", bufs=WARM + 1) as pqt,
        ):
            pqts = {}
            attnTs = {}

            def emit_B2(i):
                """q projection + elu + phi_qT for tile i."""
                if not (0 <= i < NT):
                    return
                xnT = qxnTs.pop(i)
                qh_ps = [proj_ps.tile([128, 512], F32, tag="proj",
                                      name="qh_ps")
                         for _ in range(2)]
                for kt in range(KT):
                    for c in range(2):
                        nc.tensor.matmul(
                            qh_ps[c][:], xnT[:, kt, :],
                            w_sb["q"][:, kt, c * 512:(c + 1) * 512],
                            start=(kt == 0), stop=(kt == KT - 1))
                phi_q = s2.tile([128, HS], BF16, tag="phi_q")
                for c in range(2):
                    if c_bc["q"] is not None:
                        nc.vector.tensor_tensor(
                            out=qh_ps[c][:], in0=qh_ps[c][:],
                            in1=c_bc["q"][:, c * 512:(c + 1) * 512],
                            op=ALU.add)
                    _elu1(nc, s2, qh_ps[c], phi_q[:, c * 512:(c + 1) * 512],
                          None)
                phi_qT = pqt.tile([128, KT, 128], BF16, tag="phi_qT")
                nc.sync.dma_start_transpose(out=phi_qT[:], in_=phi_q[:])
                pqts[i] = phi_qT

            def emit_C2(i):
                """nd matmuls + den/rd + attn scaling + attnT for tile i."""
                if not (0 <= i < NT):
                    return
                phi_qT = pqts.pop(i)
                nds = []
                for m in range(8):
                    if m % 2 == 0:
                        nd2 = nd_psp.tile([128, 2, 2 * (D + 1)], F32,
                                          tag="nd", name="nd",
                                          padded_shape=[128, 2, 256])
                        nds.append(nd2)
                    nc.tensor.matmul(
                        nd2[:, m % 2, :], phi_qT[:, m, :], kv2[:, m, :],
                        start=True, stop=True,
                    )
                den = stat2.tile([128, NH], F32, tag="den")
                for p in range(4):
                    nc.vector.tensor_copy(
                        out=den[:, 4 * p:4 * p + 4].rearrange(
                            "a (b c) -> a b c", b=2),
                        in_=nds[p][:, :, D::D + 1])
                rd = stat2.tile([128, NH], F32, tag="rd")
                nc.vector.tensor_scalar_add(out=rd[:], in0=den[:],
                                            scalar1=ATT_EPS)
                nc.vector.reciprocal(out=rd[:], in_=rd[:])
                attn = s2.tile([128, HS], BF16, tag="attn")
                for n in range(NH):
                    nd = nds[n // 4][:, (n // 2) % 2, :]
                    src = nd[:, (n % 2) * (D + 1):(n % 2) * (D + 1) + D]
                    if n % 2 == 0:
                        nc.scalar.activation(
                            out=attn[:, n * D:(n + 1) * D], in_=src,
                            func=AF.Copy, bias=0.0, scale=rd[:, n:n + 1])
                    else:
                        nc.vector.tensor_scalar_mul(
                            out=attn[:, n * D:(n + 1) * D], in0=src,
                            scalar1=rd[:, n:n + 1])
                attnT = s2.tile([128, KT, 128], BF16, tag="attnT")
                nc.sync.dma_start_transpose(out=attnT[:], in_=attn[:])
                attnTs[i] = attnT

            def emit_D2(i):
                """fc + residual + store for tile i."""
                if not (0 <= i < NT):
                    return
                attnT = attnTs.pop(i)
                q_nat = qloads.pop(i)
                fc_ps = [proj_ps.tile([128, 512], F32, tag="proj",
                                      name="fc_ps")
                         for _ in range(2)]
                for kt in range(KT):
                    for c in range(2):
                        nc.tensor.matmul(
                            fc_ps[c][:], attnT[:, kt, :],
                            w_sb["fc"][:, kt, c * 512:(c + 1) * 512],
                            start=(kt == 0), stop=(kt == KT - 1))
                out_sb = s2.tile([128, HS], F32, tag="out_sb")
                for c in range(2):
                    if c_bc["fc"] is not None:
                        nc.vector.tensor_tensor(
                            out=fc_ps[c][:], in0=fc_ps[c][:],
                            in1=c_bc["fc"][:, c * 512:(c + 1) * 512],
                            op=ALU.add)
                    nc.vector.tensor_tensor(
                        out=out_sb[:, c * 512:(c + 1) * 512], in0=fc_ps[c][:],
                        in1=q_nat[:, c * 512:(c + 1) * 512], op=ALU.add)
                r0 = i * 128
                nc.sync.dma_start(out=out_d[r0:r0 + 128, :], in_=out_sb[:])

            for i in range(NT + WARM + 1):
                emit_A2(i + WARM)       # tiles WARM..NT-1 (0..WARM-1 done)
                emit_B2(i)
                emit_C2(i - WARM)
                emit_D2(i - WARM - 1)

    nc.compile()
    return nc


_BUILD_CACHE = {}


def _get_nc(flags, replica_groups):
    key = (flags, tuple(tuple(g) for g in replica_groups))
    if key not in _BUILD_CACHE:
        _BUILD_CACHE[key] = build(*flags, replica_groups)
    return _BUILD_CACHE[key]


def host_prep(q, k, v, ln_q_g, ln_q_b, wq, bq, ln_k_g, ln_k_b, wk, bk,
              ln_v_g, ln_v_b, wv, bv, fc_w, fc_b, mask):
    """Fold LN gains into W, combine biases, cast to bf16. Returns
    (flags, in_maps, groups)."""
    q = np.ascontiguousarray(q, np.float32).reshape(B * S, HS)
    k = np.ascontiguousarray(k, np.float32).reshape(B * S, HS)
    v = np.ascontiguousarray(v, np.float32).reshape(B * S, HS)
    mask_f = np.ascontiguousarray(mask, np.float32).reshape(B * S, 1)

    w_eff = {}
    c_eff = {}
    for s, g, b, w, pb in (("q", ln_q_g, ln_q_b, wq, bq),
                           ("k", ln_k_g, ln_k_b, wk, bk),
                           ("v", ln_v_g, ln_v_b, wv, bv)):
        g = np.asarray(g, np.float32)
        b = np.asarray(b, np.float32)
        w = np.asarray(w, np.float32)
        pb = np.asarray(pb, np.float32)
        we = w * g[:, None] if not np.all(g == 1.0) else w
        w_eff[s] = np.ascontiguousarray(we.astype(NP_BF16))
        c_eff[s] = (b @ we + pb).astype(np.float32)
    w_eff["fc"] = np.ascontiguousarray(
        np.asarray(fc_w, np.float32).astype(NP_BF16))
    c_eff["fc"] = np.asarray(fc_b, np.float32)

    has_c = any(np.any(c != 0.0) for c in c_eff.values())
    has_mask = not bool(np.all(mask_f == 1.0))

    qb = np.ascontiguousarray(q.astype(NP_BF16))
    kb = np.ascontiguousarray(k.astype(NP_BF16))
    vb = np.ascontiguousarray(v.astype(NP_BF16))

    in_maps = []
    for c in range(NCORES):
        r0, r1 = c * TOK, (c + 1) * TOK
        m = {
            "qx": qb[r0:r1], "kx": kb[r0:r1], "vx": vb[r0:r1],
            "w_q": w_eff["q"], "w_k": w_eff["k"], "w_v": w_eff["v"],
            "w_fc": w_eff["fc"],
        }
        if has_mask:
            m["maskx"] = mask_f[r0:r1]
        if has_c:
            for s in ("q", "k", "v", "fc"):
                m[f"c_{s}"] = c_eff[s]
        in_maps.append(m)

    groups = [[0, 1], [2, 3], [4, 5], [6, 7]]
    return (has_c, has_mask), in_maps, groups


def kernel(**inputs):
    flags, in_maps, groups = host_prep(**inputs)
    nc = _get_nc(flags, groups)
    res = run_bass_kernel_spmd(nc, in_maps, list(range(NCORES)))
    out = np.concatenate([res.results[c]["out"] for c in range(NCORES)], 0)
    return out.reshape(B, S, HS).astype(np.float32)


# revision 17
# speedup vs baseline: 1.6576x; 1.0428x over previous
"""Trainium2 Bass kernel for nn_MultiHeadAttention_41936060678770.

LinBERT-style linear attention:
  qh/kh/vh = LN(x) @ W + b  (per-stream LN, 16 heads x 64 dim)
  phi = elu(.)+1 ;  phi_k masked
  kv = sum_s phi_k (x) vh ; z = sum_s phi_k
  attn = (phi_q @ kv) / (phi_q @ z + eps)
  out = q + attn @ fc_w + fc_b

Sharding: 8 cores, tokens split 8-ways over flattened (B*S); each pair of
cores (2c, 2c+1) holds one batch, so the [16,64,65] kv/z state is
all-reduced within core pairs; everything else is fully local.

v4 (745us baseline -> 526 -> 485 -> this). Trace-driven changes:
  - host folds LN gain into W, precomputes combined bias, casts weights
    and q/k/v to bf16 (HBM input traffic 42MB -> 21MB);
  - weights load via the otherwise-idle gpsimd queue (one DMA each) so
    the sync queue's ~0.7us/launch budget goes to activation tiles; the
    485us trace lost 53us at startup to weight/load launch contention;
  - all 16 q tiles load + LN + transpose during sweep1's tail (their
    queues have slack), so sweep2's projections start the moment
    sweep1's PE work drains and the kv AllReduce is fully hidden behind
    a 7-tile projection warmup;
  - vh_aug copies and the residual adds run on gpsimd (Pool), exp/xn/
    half the attn scales on ACT, stats/newton/elu-min/stt/den on DVE:
    every engine stays under the PE's ~9.4us/tile;
  - PE p-state: dense back-to-back matmuls ramp the PE to 2.4GHz; idle
    gaps drop it to 1.2GHz, so the whole design minimizes PE stalls.
"""
import sys

sys.path.insert(0, "/opt/trn_rl_repo")

import ml_dtypes
import numpy as np

import concourse.bacc as bacc
import concourse.bass as bass
import concourse.tile as tile
import concourse.mybir as mybir
from concourse.bass_utils import run_bass_kernel_spmd

F32 = mybir.dt.float32
BF16 = mybir.dt.bfloat16
NP_BF16 = ml_dtypes.bfloat16
AF = mybir.ActivationFunctionType
ALU = mybir.AluOpType

B, S, HS = 4, 4096, 1024
NH, D = 16, 64
NCORES = 8
TOK = B * S // NCORES          # 2048 rows per core
NT = TOK // 128                # 16 token tiles
KT = HS // 128                 # 8 hidden tiles
LN_EPS = 1e-5
ATT_EPS = 1e-6
PF = 3                         # sweep1 k/v load prefetch depth (tiles)
WARM = 7                       # sweep2 projection warmup depth (hides CC)


def _ln_stats(nc, stat_pool, x_nat):
    """bn stats + rsqrt(var+eps) via DVE newton. Returns (sig, negmusig)."""
    mv = stat_pool.tile([128, 2], F32, tag="mv")
    stats = stat_pool.tile([128, 2, 6], F32, tag="stats")
    nc.vector.bn_stats(out=stats[:, 0, :], in_=x_nat[:, 0:512])
    nc.vector.bn_stats(out=stats[:, 1, :], in_=x_nat[:, 512:1024])
    nc.vector.bn_aggr(out=mv[:], in_=stats[:])
    # rsig = rsqrt(var+eps) fully on DVE (quake seed + 2 Newton steps) so
    # ACT only ever runs one table set.
    veps = stat_pool.tile([128, 1], F32, tag="veps")
    nc.vector.tensor_scalar_add(out=veps[:], in0=mv[:, 1:2], scalar1=LN_EPS)
    seed = stat_pool.tile([128, 1], mybir.dt.int32, tag="seed")
    nc.vector.tensor_scalar(
        out=seed[:], in0=veps[:].bitcast(mybir.dt.int32),
        scalar1=1, scalar2=None, op0=ALU.arith_shift_right)
    nc.vector.tensor_scalar(
        out=seed[:], in0=seed[:], scalar1=-1, scalar2=0x5F3759DF,
        op0=ALU.mult, op1=ALU.add)
    y0 = seed[:].bitcast(F32)
    t_nr = stat_pool.tile([128, 1], F32, tag="t_nr")
    sig = stat_pool.tile([128, 1], F32, tag="sig")
    nc.vector.tensor_tensor(out=t_nr[:], in0=y0, in1=y0, op=ALU.mult)
    nc.vector.tensor_tensor(out=t_nr[:], in0=t_nr[:], in1=veps[:],
                            op=ALU.mult)
    nc.vector.tensor_scalar(out=t_nr[:], in0=t_nr[:], scalar1=-0.5,
                            scalar2=1.5, op0=ALU.mult, op1=ALU.add)
    nc.vector.tensor_tensor(out=sig[:], in0=y0, in1=t_nr[:], op=ALU.mult)
    nc.vector.tensor_tensor(out=t_nr[:], in0=sig[:], in1=sig[:], op=ALU.mult)
    nc.vector.tensor_tensor(out=t_nr[:], in0=t_nr[:], in1=veps[:],
                            op=ALU.mult)
    nc.vector.tensor_scalar(out=t_nr[:], in0=t_nr[:], scalar1=-0.5,
                            scalar2=1.5, op0=ALU.mult, op1=ALU.add)
    nc.vector.tensor_tensor(out=sig[:], in0=sig[:], in1=t_nr[:], op=ALU.mult)
    negmusig = stat_pool.tile([128, 1], F32, tag="negmusig")
    nc.vector.scalar_tensor_tensor(
        out=negmusig[:], in0=mv[:, 0:1], scalar=-1.0, in1=sig[:],
        op0=ALU.mult, op1=ALU.mult)
    return sig, negmusig


def _elu1(nc, pool, src_ps, out_ap, mask_col):
    """out = elu(src)+1 = max(src,0) + min(exp(src),1), optionally * mask.
    exp reads PSUM directly on ACT (values here never overflow exp)."""
    texp = pool.tile([128, 512], BF16, tag="texp")
    nc.scalar.activation(out=texp[:], in_=src_ps[:], func=AF.Exp)
    tmin = pool.tile([128, 512], BF16, tag="tmin")
    nc.vector.tensor_scalar_min(out=tmin[:], in0=texp[:], scalar1=1.0)
    if mask_col is None:
        nc.vector.scalar_tensor_tensor(
            out=out_ap, in0=src_ps[:], scalar=0.0, in1=tmin[:],
            op0=ALU.max, op1=ALU.add,
        )
    else:
        tphi = pool.tile([128, 512], F32, tag="tphi")
        nc.vector.scalar_tensor_tensor(
            out=tphi[:], in0=src_ps[:], scalar=0.0, in1=tmin[:],
            op0=ALU.max, op1=ALU.add,
        )
        nc.vector.tensor_scalar_mul(out=out_ap, in0=tphi[:], scalar1=mask_col)


def build(has_c: bool, has_mask: bool, replica_groups,
          _skip_collective=False):
    nc = bacc.Bacc(None)

    qx_d = nc.dram_tensor("qx", [TOK, HS], BF16, kind="ExternalInput")
    kx_d = nc.dram_tensor("kx", [TOK, HS], BF16, kind="ExternalInput")
    vx_d = nc.dram_tensor("vx", [TOK, HS], BF16, kind="ExternalInput")
    w_d = {s: nc.dram_tensor(f"w_{s}", [HS, HS], BF16, kind="ExternalInput")
           for s in ("q", "k", "v", "fc")}
    if has_mask:
        mask_d = nc.dram_tensor("maskx", [TOK, 1], F32, kind="ExternalInput")
    if has_c:
        c_d = {s: nc.dram_tensor(f"c_{s}", [HS], F32, kind="ExternalInput")
               for s in ("q", "k", "v", "fc")}

    out_d = nc.dram_tensor("out", [TOK, HS], F32, kind="ExternalOutput")

    from contextlib import ExitStack
    with tile.TileContext(nc) as tc, ExitStack() as ctx:
        wpool = ctx.enter_context(tc.tile_pool(name="weights", bufs=1))
        consts = ctx.enter_context(tc.tile_pool(name="consts", bufs=1))
        dram_p = ctx.enter_context(
            tc.tile_pool(name="dram", bufs=1, space="DRAM"))
        # q-side pools live across both sweeps (q prep happens in sweep1)
        q_ld = ctx.enter_context(tc.tile_pool(name="q_ld", bufs=NT))
        qw_xn = ctx.enter_context(tc.tile_pool(name="qw_xn", bufs=3))
        qw_xnT = ctx.enter_context(tc.tile_pool(name="qw_xnT", bufs=WARM + 2))
        stat2 = ctx.enter_context(tc.tile_pool(name="stat2", bufs=8))

        # ---------------- weights (bf16 from host, gpsimd queue) ----------
        w_sb = {}
        for s in ("k", "v", "q", "fc"):
            w_sb[s] = wpool.tile([128, KT, HS], BF16, tag=f"w_{s}",
                                 name=f"w_{s}")
            nc.gpsimd.dma_start(
                out=w_sb[s][:],
                in_=w_d[s].rearrange("(kt p) n -> p kt n", p=128))

        c_bc = {"q": None, "k": None, "v": None, "fc": None}
        if has_c:
            for s in ("q", "k", "v", "fc"):
                crow = consts.tile([1, HS], F32, tag=f"crow_{s}")
                nc.sync.dma_start(out=crow[:], in_=c_d[s][None, :])
                c_bc[s] = consts.tile([128, HS], F32, tag=f"cbc_{s}",
                                      name=f"cbc_{s}")
                nc.gpsimd.partition_broadcast(c_bc[s][:], crow[:])

        qloads = {}
        qxnTs = {}

        def emit_load2(i):
            if not (0 <= i < NT):
                return
            r0 = i * 128
            q_nat = q_ld.tile([128, HS], BF16, tag="q_nat")
            nc.sync.dma_start(out=q_nat[:], in_=qx_d[r0:r0 + 128, :])
            qloads[i] = q_nat

        def emit_A2(i):
            """LN + xn + transpose of q tile i (runs during sweep1 tail)."""
            if not (0 <= i < NT):
                return
            q_nat = qloads[i]
            sig, negmusig = _ln_stats(nc, stat2, q_nat)
            xn = qw_xn.tile([128, HS], BF16, tag="xn_q")
            nc.scalar.activation(out=xn[:], in_=q_nat[:], func=AF.Identity,
                                 scale=sig[:], bias=negmusig[:])
            xnT = qw_xnT.tile([128, KT, 128], BF16, tag="xnT_q")
            nc.sync.dma_start_transpose(out=xnT[:], in_=xn[:])
            qxnTs[i] = xnT

        # ---------------- sweep 1: K/V + kv state ----------------
        kv_sb = consts.tile([128, 8, D + 1], F32, tag="kv_sb")
        with (
            tc.tile_pool(name="kv_ps", bufs=1, space="PSUM") as kv_psp,
            tc.tile_pool(name="kh_ps", bufs=3, space="PSUM") as kh_psp,
            tc.tile_pool(name="vh_ps", bufs=3, space="PSUM") as vh_psp,
            tc.tile_pool(name="s1", bufs=3) as s1,
            tc.tile_pool(name="ld1", bufs=6) as ld1,
            tc.tile_pool(name="stat1", bufs=8) as stat1,
        ):
            kv_ps = [kv_psp.tile([128, 4, D + 1], F32, tag=f"kv{b}",
                                 name=f"kv{b}", padded_shape=[128, 4, 128])
                     for b in range(2)]

            loads = {}

            def emit_load1(i):
                if i >= NT:
                    return
                r0 = i * 128
                k_nat = ld1.tile([128, HS], BF16, tag="k_nat")
                nc.sync.dma_start(out=k_nat[:], in_=kx_d[r0:r0 + 128, :])
                v_nat = ld1.tile([128, HS], BF16, tag="v_nat")
                nc.sync.dma_start(out=v_nat[:], in_=vx_d[r0:r0 + 128, :])
                mask_col = None
                if has_mask:
                    mcol = stat1.tile([128, 1], F32, tag="mcol")
                    nc.sync.dma_start(out=mcol[:], in_=mask_d[r0:r0 + 128, :])
                    mask_col = mcol[:]
                loads[i] = (k_nat, v_nat, mask_col)

            def emit_A1(i):
                """LN + xn + transpose for k and v of tile i."""
                if i >= NT:
                    return None
                k_nat, v_nat, mask_col = loads.pop(i)
                res = {}
                for s, x_nat in (("k", k_nat), ("v", v_nat)):
                    sig, negmusig = _ln_stats(nc, stat1, x_nat)
                    xn = s1.tile([128, HS], BF16, tag=f"xn_{s}")
                    nc.scalar.activation(out=xn[:], in_=x_nat[:],
                                         func=AF.Identity,
                                         scale=sig[:], bias=negmusig[:])
                    xnT = s1.tile([128, KT, 128], BF16, tag=f"xnT_{s}")
                    nc.sync.dma_start_transpose(out=xnT[:], in_=xn[:])
                    res[s] = xnT
                res["mask"] = mask_col
                return res

            def emit_B1(i, a):
                """proj k,v + elu(k) + vh_aug for tile i -> (phi_k, vh_aug)."""
                if a is None:
                    return None
                kh_ps = [kh_psp.tile([128, 512], F32, tag="proj",
                                     name="kh_ps")
                         for _ in range(2)]
                for kt in range(KT):
                    for c in range(2):
                        nc.tensor.matmul(
                            kh_ps[c][:], a["k"][:, kt, :],
                            w_sb["k"][:, kt, c * 512:(c + 1) * 512],
                            start=(kt == 0), stop=(kt == KT - 1))
                vh_ps = [vh_psp.tile([128, 512], F32, tag="proj",
                                     name="vh_ps")
                         for _ in range(2)]
                for kt in range(KT):
                    for c in range(2):
                        nc.tensor.matmul(
                            vh_ps[c][:], a["v"][:, kt, :],
                            w_sb["v"][:, kt, c * 512:(c + 1) * 512],
                            start=(kt == 0), stop=(kt == KT - 1))
                phi_k = s1.tile([128, HS], BF16, tag="phi_k")
                for c in range(2):
                    if c_bc["k"] is not None:
                        nc.vector.tensor_tensor(
                            out=kh_ps[c][:], in0=kh_ps[c][:],
                            in1=c_bc["k"][:, c * 512:(c + 1) * 512],
                            op=ALU.add)
                    _elu1(nc, s1, kh_ps[c], phi_k[:, c * 512:(c + 1) * 512],
                          a["mask"])
                vh_aug = s1.tile([128, NH, D + 1], BF16, tag="vh_aug")
                nc.gpsimd.memset(vh_aug[:, :, D:D + 1], 1.0)
                for c in range(2):
                    if c_bc["v"] is not None:
                        nc.vector.tensor_tensor(
                            out=vh_ps[c][:], in0=vh_ps[c][:],
                            in1=c_bc["v"][:, c * 512:(c + 1) * 512],
                            op=ALU.add)
                    nc.scalar.activation(
                        out=vh_aug[:, c * 8:(c + 1) * 8, 0:D],
                        in_=vh_ps[c][:].rearrange("p (n d) -> p n d", d=D),
                        func=AF.Copy)
                return phi_k, vh_aug

            def emit_kv(i, b):
                if b is None:
                    return
                phi_k, vh_aug = b
                for n in range(NH):
                    beta, j, hs = n // 8, (n // 2) % 4, (n % 2) * 64
                    nc.tensor.matmul(
                        kv_ps[beta][hs:hs + 64, j, :],
                        phi_k[:, n * D:(n + 1) * D],
                        vh_aug[:, n, :],
                        start=(i == 0), stop=(i == NT - 1),
                        tile_position=(0, hs),
                        skip_group_check=True,
                    )

            for j in range(PF):
                emit_load1(j)
            a_cur = emit_A1(0)
            b_prev = None
            for i in range(NT):
                emit_load1(i + PF)
                # q prep rides sweep1's spare DVE/ACT/sync capacity:
                # all q loads early, LN+transpose for the first WARM tiles.
                emit_load2(i)
                if i >= NT - WARM:
                    emit_A2(i - (NT - WARM))
                a_next = emit_A1(i + 1)
                b_cur = emit_B1(i, a_cur)
                emit_kv(i - 1, b_prev)
                a_cur, b_prev = a_next, b_cur
            emit_kv(NT - 1, b_prev)

            nc.vector.tensor_copy(out=kv_sb[:, 0:4, :], in_=kv_ps[0][:])
            nc.vector.tensor_copy(out=kv_sb[:, 4:8, :], in_=kv_ps[1][:])

        # ---------------- all-reduce kv state within batch pairs ----------
        # kv2 holds the reduced state as 8 block-diagonal [128, 130] bf16
        # operands (head-pair 2m/2m+1), so the num/den matmul is a plain
        # K=128 matmul at base partition 0.
        kv2 = consts.tile([128, 8, 2 * (D + 1)], BF16, tag="kv2")
        nc.vector.memset(kv2[:], 0.0)
        if _skip_collective:
            nc.vector.tensor_copy(out=kv2[0:64, :, 0:D + 1],
                                  in_=kv_sb[0:64, :, :])
            nc.vector.tensor_copy(out=kv2[64:128, :, D + 1:2 * (D + 1)],
                                  in_=kv_sb[64:128, :, :])
        else:
            cc_in = dram_p.tile([128, 8, D + 1], F32)
            cc_out = dram_p.tile([128, 8, D + 1], F32)
            nc.gpsimd.dma_start(out=cc_in[:], in_=kv_sb[:])
            nc.gpsimd.collective_compute(
                "AllReduce", ALU.add, replica_groups=replica_groups,
                ins=[cc_in.opt()], outs=[cc_out.opt()],
            )
            nc.gpsimd.dma_start(out=kv2[0:64, :, 0:D + 1],
                                in_=cc_out[0:64, :, :])
            nc.gpsimd.dma_start(out=kv2[64:128, :, D + 1:2 * (D + 1)],
                                in_=cc_out[64:128, :, :])

        # ---------------- sweep 2: Q -> attn -> fc -> out ----------------
        with (
            tc.tile_pool(name="proj_ps", bufs=4, space="PSUM") as proj_ps,
            tc.tile_pool(name="nd_ps", bufs=4, space="PSUM") as nd_psp,
            tc.tile_pool(name="s2", bufs=4) as s2,
            tc.tile_pool(name="pqt", bufs=WARM + 1) as pqt,
        ):
            pqts = {}
            attnTs = {}

            def emit_B2(i):
                """q projection + elu + phi_qT for tile i."""
                if not (0 <= i < NT):
                    return
                xnT = qxnTs.pop(i)
                qh_ps = [proj_ps.tile([128, 512], F32, tag="proj",
                                      name="qh_ps")
                         for _ in range(2)]
                for kt in range(KT):
                    for c in range(2):
                        nc.tensor.matmul(
                            qh_ps[c][:], xnT[:, kt, :],
                            w_sb["q"][:, kt, c * 512:(c + 1) * 512],
                            start=(kt == 0), stop=(kt == KT - 1))
                phi_q = s2.tile([128, HS], BF16, tag="phi_q")
                for c in range(2):
                    if c_bc["q"] is not None:
                        nc.vector.tensor_tensor(
                            out=qh_ps[c][:], in0=qh_ps[c][:],
                            in1=c_bc["q"][:, c * 512:(c + 1) * 512],
                            op=ALU.add)
                    _elu1(nc, s2, qh_ps[c], phi_q[:, c * 512:(c + 1) * 512],
                          None)
                phi_qT = pqt.tile([128, KT, 128], BF16, tag="phi_qT")
                nc.sync.dma_start_transpose(out=phi_qT[:], in_=phi_q[:])
                pqts[i] = phi_qT

            def emit_C2(i):
                """nd matmuls + den/rd + attn scaling + attnT for tile i."""
                if not (0 <= i < NT):
                    return
                phi_qT = pqts.pop(i)
                nds = []
                for m in range(8):
                    if m % 2 == 0:
                        nd2 = nd_psp.tile([128, 2, 2 * (D + 1)], F32,
                                          tag="nd", name="nd",
                                          padded_shape=[128, 2, 256])
                        nds.append(nd2)
                    nc.tensor.matmul(
                        nd2[:, m % 2, :], phi_qT[:, m, :], kv2[:, m, :],
                        start=True, stop=True,
                    )
                den = stat2.tile([128, NH], F32, tag="den")
                for p in range(4):
                    nc.vector.tensor_copy(
                        out=den[:, 4 * p:4 * p + 4].rearrange(
                            "a (b c) -> a b c", b=2),
                        in_=nds[p][:, :, D::D + 1])
                rd = stat2.tile([128, NH], F32, tag="rd")
                nc.vector.tensor_scalar_add(out=rd[:], in0=den[:],
                                            scalar1=ATT_EPS)
                nc.vector.reciprocal(out=rd[:], in_=rd[:])
                attn = s2.tile([128, HS], BF16, tag="attn")
                for n in range(NH):
                    nd = nds[n // 4][:, (n // 2) % 2, :]
                    src = nd[:, (n % 2) * (D + 1):(n % 2) * (D + 1) + D]
                    if n % 2 == 0:
                        nc.scalar.activation(
                            out=attn[:, n * D:(n + 1) * D], in_=src,
                            func=AF.Copy, bias=0.0, scale=rd[:, n:n + 1])
                    else:
                        nc.vector.tensor_scalar_mul(
                            out=attn[:, n * D:(n + 1) * D], in0=src,
                            scalar1=rd[:, n:n + 1])
                attnT = s2.tile([128, KT, 128], BF16, tag="attnT")
                nc.sync.dma_start_transpose(out=attnT[:], in_=attn[:])
                attnTs[i] = attnT

            def emit_D2(i):
                """fc + residual + store for tile i."""
                if not (0 <= i < NT):
                    return
                attnT = attnTs.pop(i)
                q_nat = qloads.pop(i)
                fc_ps = [proj_ps.tile([128, 512], F32, tag="proj",
                                      name="fc_ps")
                         for _ in range(2)]
                for kt in range(KT):
                    for c in range(2):
                        nc.tensor.matmul(
                            fc_ps[c][:], attnT[:, kt, :],
                            w_sb["fc"][:, kt, c * 512:(c + 1) * 512],
                            start=(kt == 0), stop=(kt == KT - 1))
                out_sb = s2.tile([128, HS], F32, tag="out_sb")
                for c in range(2):
                    if c_bc["fc"] is not None:
                        nc.vector.tensor_tensor(
                            out=fc_ps[c][:], in0=fc_ps[c][:],
                            in1=c_bc["fc"][:, c * 512:(c + 1) * 512],
                            op=ALU.add)
                    nc.vector.tensor_tensor(
                        out=out_sb[:, c * 512:(c + 1) * 512], in0=fc_ps[c][:],
                        in1=q_nat[:, c * 512:(c + 1) * 512], op=ALU.add)
                r0 = i * 128
                nc.sync.dma_start(out=out_d[r0:r0 + 128, :], in_=out_sb[:])

            for i in range(NT + WARM + 1):
                emit_A2(i + WARM)       # tiles WARM..NT-1 (0..WARM-1 done)
                emit_B2(i)
                emit_C2(i - WARM)
                emit_D2(i - WARM - 1)

    nc.compile()
    return nc


_BUILD_CACHE = {}


def _get_nc(flags, replica_groups):
    key = (flags, tuple(tuple(g) for g in replica_groups))
    if key not in _BUILD_CACHE:
        _BUILD_CACHE[key] = build(*flags, replica_groups)
    return _BUILD_CACHE[key]


def host_prep(q, k, v, ln_q_g, ln_q_b, wq, bq, ln_k_g, ln_k_b, wk, bk,
              ln_v_g, ln_v_b, wv, bv, fc_w, fc_b, mask):
    """Fold LN gains into W, combine biases, cast to bf16. Returns
    (flags, in_maps, groups)."""
    q = np.ascontiguousarray(q, np.float32).reshape(B * S, HS)
    k = np.ascontiguousarray(k, np.float32).reshape(B * S, HS)
    v = np.ascontiguousarray(v, np.float32).reshape(B * S, HS)
    mask_f = np.ascontiguousarray(mask, np.float32).reshape(B * S, 1)

    w_eff = {}
    c_eff = {}
    for s, g, b, w, pb in (("q", ln_q_g, ln_q_b, wq, bq),
                           ("k", ln_k_g, ln_k_b, wk, bk),
                           ("v", ln_v_g, ln_v_b, wv, bv)):
        g = np.asarray(g, np.float32)
        b = np.asarray(b, np.float32)
        w = np.asarray(w, np.float32)
        pb = np.asarray(pb, np.float32)
        we = w * g[:, None] if not np.all(g == 1.0) else w
        w_eff[s] = np.ascontiguousarray(we.astype(NP_BF16))
        c_eff[s] = (b @ we + pb).astype(np.float32)
    w_eff["fc"] = np.ascontiguousarray(
        np.asarray(fc_w, np.float32).astype(NP_BF16))
    c_eff["fc"] = np.asarray(fc_b, np.float32)

    has_c = any(np.any(c != 0.0) for c in c_eff.values())
    has_mask = not bool(np.all(mask_f == 1.0))

    qb = np.ascontiguousarray(q.astype(NP_BF16))
    kb = np.ascontiguousarray(k.astype(NP_BF16))
    vb = np.ascontiguousarray(v.astype(NP_BF16))

    in_maps = []
    for c in range(NCORES):
        r0, r1 = c * TOK, (c + 1) * TOK
        m = {
            "qx": qb[r0:r1], "kx": kb[r0:r1], "vx": vb[r0:r1],
            "w_q": w_eff["q"], "w_k": w_eff["k"], "w_v": w_eff["v"],
            "w_fc": w_eff["fc"],
        }
        if has_mask:
            m["maskx"] = mask_f[r0:r1]
        if has_c:
            for s in ("q", "k", "v", "fc"):
                m[f"c_{s}"] = c_eff[s]
        in_maps.append(m)

    groups = [[0, 1], [2, 3], [4, 5], [6, 7]]
    return (has_c, has_mask), in_maps, groups


def kernel(**inputs):
    flags, in_maps, groups = host_prep(**inputs)
    nc = _get_nc(flags, groups)
    res = run_bass_kernel_spmd(nc, in_maps, list(range(NCORES)))
    out = np.concatenate([res.results[c]["out"] for c in range(NCORES)], 0)
    return out.reshape(B, S, HS).astype(np.float32)
